# revision 47
# baseline (speedup 1.0000x reference)
"""NostARHead attention kernel for Trainium2 (8 NeuronCores, batch-parallel).

Strategy
--------
Data-parallel over batch: core b handles batch element b (B == n_cores == 8).

Algebraic structure exploited:
  1. The single query token sits at rotary position 0, where the RoPE
     rotation is the identity (sin 0 = 0, cos 0 = 1).  The attention score
     therefore factors through a fixed basis of cos/sin position features:
        score[t,h] = sum_j cos(th_j t) * a[t,h,j] + sin(th_j t) * b[t,h,j]
                     + plain[t,h]
     where a, b, plain are LINEAR in hs[t] with weights combining q and W_k.
     Since the last-token index (and hence q) is host-computable, the host
     builds a fused score-projection matrix W_sc = [A | B | plain] of shape
     [E, 2*H*R2 + H] = [2048, 1040] -- roughly HALF the FLOPs of the full
     K-projection, with no on-device RoPE and no on-device q-path at all.
     W_sc is fp16 (10-bit mantissa; scores accumulate in fp32 PSUM).
  2. LayerNorm's per-token affine commutes with everything downstream:
     the device transposes and pools the RAW hidden states and applies the
     (r_t, mu_t) correction to the 16 scores per token (score path) and as
     a rank-1 correction kappa to the pooled vector (value path):
        score[t]  = r_t * combo_raw[t] - (r_t mu_t) * corr_s[t]   (+corr_b)
        z[h]      = (sum_t es[t,h] r_t hs_raw[t] - kappa[h] 1) / D_h
     corr_s is a host-precomputed [S,H] table.  r_t = rsqrt(var+eps) is a
     3-step Newton iteration on GPSIMD (var is within a few percent of 1),
     keeping the ACT engine exp-only (no activation-table reloads) and the
     serial chain off the DVE queue.
  3. With a single query the V-projection commutes with attention pooling:
     pool first (z = es^T @ hs_raw, an fp32r [H,E] matmul against the raw
     tile), then apply W_v per head (bf16), then the out-projection (bf16).
     kappa and the softmax denominator come from one extra matmul
     es2^T @ [mu | sd] accumulated in a shared PSUM bank whose has_written
     bits are never cleared after t=0 (plain regions are per-tile columns).

Scheduling
----------
Per 128-token tile, software-pipelined: PE runs [transpose-group(t+1) |
score-chunk(t)] x4 then the pooling matmuls of t-1 (so it never waits on
the DVE combine); ACT drains transpose PSUM and runs exp; DVE runs
bn_stats and the cos/sin combine; GPSIMD runs the rsqrt chain.  All DMA is
issued on one explicitly-ordered SP stream (hidden-state tiles ahead of
the paced vw/ow prefetch; pool ring slots provide flow control).  Tiles
t >= 2 are downcast to fp16 on ACT so their transposes run at 1 cycle/row.
The output is copied and DMA'd per 256-column chunk as the out-projection
completes.

The module compiles the program once (shapes are static) and caches it.
"""

import numpy as np
import ml_dtypes

import concourse.bass as bass
import concourse.mybir as mybir
import concourse.tile as tile
from concourse import bacc, bass_utils
from concourse.masks import make_identity

F32 = mybir.dt.float32
F32R = mybir.dt.float32r
F16 = mybir.dt.float16
BF16 = mybir.dt.bfloat16

P = 128
B = 8
S = 2048
E = 2048
H = 16
D = 128
ROT = 64
R2 = ROT // 2        # 32 rotary pairs per head
PAD = 50257
EPS = 1e-5

EC = E // P          # 16 feature chunks
TC = S // P          # 16 token tiles
NA = H * R2          # 512 cols in the A (cos) block
NB = H * R2          # 512 cols in the B (sin) block
NW = 256             # weight free-dim slice for the tail projections
NO = E // NW         # 8 output-dim slices
HPW = NW // D        # heads per weight slice (2)

_CACHE = {}


def _build_program(flags):
    """Per-core SPMD program. flags: (has_corrb, has_vbias, has_obias)"""
    has_corrb, has_vbias, has_obias = flags
    nc = bacc.Bacc("TRN2", debug=False, num_devices=B)

    in_hs = nc.dram_tensor("hs", [S, E], F32R, kind="ExternalInput").ap()
    in_wa = nc.dram_tensor("wsa", [E, NA], F16, kind="ExternalInput").ap()
    in_wb = nc.dram_tensor("wsb", [E, NB], F16, kind="ExternalInput").ap()
    in_wp = nc.dram_tensor("wsp", [E, H], F16, kind="ExternalInput").ap()
    in_vw = nc.dram_tensor("vwT", [E, E], BF16, kind="ExternalInput").ap()
    in_ow = nc.dram_tensor("owT", [E, E], BF16, kind="ExternalInput").ap()
    in_ct = nc.dram_tensor("ctab", [S, R2], F32, kind="ExternalInput").ap()
    in_st = nc.dram_tensor("stab", [S, R2], F32, kind="ExternalInput").ap()
    in_co = nc.dram_tensor("corr", [S, H], F32, kind="ExternalInput").ap()
    in_cb = in_vb = in_ob = None
    if has_corrb:
        in_cb = nc.dram_tensor("corrb", [S, H], F32, kind="ExternalInput").ap()
    if has_vbias:
        in_vb = nc.dram_tensor("vbiasT", [P, EC], F32, kind="ExternalInput").ap()
    if has_obias:
        in_ob = nc.dram_tensor("obias", [1, E], F32, kind="ExternalInput").ap()
    out_t = nc.dram_tensor("out", [1, E], F32, kind="ExternalOutput").ap()

    with tile.TileContext(nc) as tc:
        with (
            tc.tile_pool(name="sing", bufs=1) as sing,
            tc.tile_pool(name="wvp", bufs=12) as wvp,
        ):
            # ---------------- constants / preloads ----------------
            ident32 = sing.tile([P, P], F32)
            make_identity(nc, ident32[:])
            ident = sing.tile([P, P], F32R)
            nc.vector.tensor_copy(out=ident[:], in_=ident32[:])
            ident_bf = sing.tile([P, P], BF16)
            nc.vector.tensor_copy(out=ident_bf[:], in_=ident32[:])
            ident_h = sing.tile([P, P], F16)
            nc.vector.tensor_copy(out=ident_h[:], in_=ident32[:])
            ctab = sing.tile([P, TC, R2], F32)
            stab = sing.tile([P, TC, R2], F32)
            corr = sing.tile([P, TC, H], F32)
            corrb = None
            if has_corrb:
                corrb = sing.tile([P, TC, H], F32)
                nc.sync.dma_start(corrb[:], in_cb.rearrange("(t p) h -> p t h", p=P))
            # fused score-projection weights, fully resident
            wsa = sing.tile([P, EC, NA], F16)
            wsb = sing.tile([P, EC, NB], F16)
            wsp = sing.tile([P, EC, H], F16)
            # prefetch the value/out projection weights; DMAs are queued
            # behind the score weights and stream in during the main loop.
            vw_tiles = []
            ow_tiles = []

            def prefetch_tail_weight(k):
                """One vw/ow chunk DMA, paced from inside the main loop so the
                prefetch never starves the hidden-state stream."""
                if k < NO:
                    vwt = wvp.tile([P, EC, NW], BF16, tag="wv", name=f"vw{k}")
                    nc.sync.dma_start(
                        vwt[:],
                        in_vw.rearrange("(ec p) o -> p ec o", p=P)[
                            :, :, k * NW:(k + 1) * NW
                        ],
                    )
                    vw_tiles.append(vwt)
                elif k < 2 * NO:
                    o = k - NO
                    owt = wvp.tile([P, EC, NW], BF16, tag="wv", name=f"ow{o}")
                    nc.sync.dma_start(
                        owt[:],
                        in_ow.rearrange("(ec p) o -> p ec o", p=P)[
                            :, :, o * NW:(o + 1) * NW
                        ],
                    )
                    ow_tiles.append(owt)

            # ---------------- main loop ----------------
            with tc.tile_pool(name="zps", bufs=1, space="PSUM") as zps:
                z_ps = [
                    zps.tile([H, 512], F32, tag=f"z{i}", name=f"z{i}")
                    for i in range(4)
                ]
                # one shared bank: 16 per-tile plain regions + [kappa|D].
                # Only two matmuls ever carry start=True in this bank (plain
                # t=0 and kd t=0), so the bank-wide has_written clear cannot
                # wipe the persistent kd accumulation.
                acc = zps.tile([P, 512], F32, tag="acc", name="acc")
                kd = acc[:H, 256:258]       # [kappa | D] accumulators (persistent)

                with (
                    tc.tile_pool(name="xtp", bufs=4) as xtp,
                    tc.tile_pool(name="lnp", bufs=3) as lnp,
                    tc.tile_pool(name="hstp", bufs=2) as hstp,
                    tc.tile_pool(name="x16p", bufs=3) as x16p,
                    tc.tile_pool(name="cmb", bufs=3) as cmb,
                    tc.tile_pool(name="cmbm", bufs=2) as cmbm,
                    tc.tile_pool(name="ptp", bufs=1, space="PSUM") as ptp,
                    tc.tile_pool(name="scp", bufs=1, space="PSUM") as scp,
                ):
                    xts = [None] * TC
                    x16s = [None] * TC
                    lns = [None] * TC
                    hsts = [None] * TC
                    ess = [None] * TC

                    def load(t):
                        xt = xtp.tile([P, E], F32R, tag="xt", name=f"xt{t}")
                        nc.sync.dma_start(xt[:], in_hs[t * P:(t + 1) * P, :])
                        xts[t] = xt

                    def ln_stats(t):
                        xt = xts[t]
                        stats = lnp.tile([P, 4, 6], F32, tag="st", name=f"st{t}")
                        for g in range(4):
                            nc.vector.bn_stats(
                                out=stats[:, g, :], in_=xt[:, g * 512:(g + 1) * 512].bitcast(F32)
                            )
                        mv = lnp.tile([P, 2], F32, tag="mv", name=f"mv{t}")
                        nc.vector.bn_aggr(out=mv[:], in_=stats[:])
                        # r = rsqrt(var + EPS) via Newton on DVE (keeps the
                        # ACT engine exp-only, avoiding act-table reloads).
                        # var is within a few percent of 1, so y0 = 1 and
                        # three iterations reach float32 roundoff.
                        # The whole serial rsqrt chain runs on the
                        # otherwise-idle GPSIMD engine: a dependent chain of
                        # tiny ops would convoy the DVE's in-order queue and
                        # stall the score combine behind it.
                        var = mv[:, 1:2]
                        vpe = lnp.tile([P, 1], F32, tag="vpe", name=f"vpe{t}")
                        r = lnp.tile([P, 1], F32, tag="r", name=f"r{t}")
                        tq = lnp.tile([P, 1], F32, tag="tq", name=f"tq{t}")
                        uq = lnp.tile([P, 1], F32, tag="uq", name=f"uq{t}")
                        nc.gpsimd.tensor_scalar_add(vpe[:], var, float(EPS))
                        nc.gpsimd.tensor_scalar_mul(r[:], vpe[:], -0.5)
                        nc.gpsimd.tensor_scalar_add(r[:], r[:], 1.5)
                        for _ in range(2):
                            nc.gpsimd.tensor_tensor(
                                tq[:], r[:], r[:], mybir.AluOpType.mult
                            )
                            nc.gpsimd.tensor_tensor(
                                uq[:], vpe[:], tq[:], mybir.AluOpType.mult
                            )
                            nc.gpsimd.tensor_scalar_mul(uq[:], uq[:], -0.5)
                            nc.gpsimd.tensor_scalar_add(uq[:], uq[:], 1.5)
                            nc.gpsimd.tensor_tensor(
                                r[:], r[:], uq[:], mybir.AluOpType.mult
                            )
                        mr = lnp.tile([P, 1], F32, tag="mr", name=f"mr{t}")
                        nc.gpsimd.tensor_tensor(
                            mr[:], mv[:, 0:1], r[:], mybir.AluOpType.mult
                        )
                        # msd = [mu | sd];  sd = (var+EPS) * r  (DVE: its F32R
                        # write must come from a rounding engine op)
                        msd = lnp.tile([P, 2], F32R, tag="msd", name=f"msd{t}")
                        nc.vector.tensor_copy(out=msd[:, 0:1], in_=mv[:, 0:1])
                        nc.vector.tensor_tensor(
                            msd[:, 1:2], vpe[:], r[:], mybir.AluOpType.mult
                        )
                        lns[t] = (mv, r, mr, msd)

                    def convert16(t):
                        """Downcast tile t to fp16 (ACT) so its transposes run
                        at 1 cycle/row instead of fp32r's 1.5."""
                        x16 = x16p.tile([P, E], F16, tag="x16", name=f"x16_{t}")
                        nc.scalar.copy(out=x16[:], in_=xts[t][:].bitcast(F32))
                        x16s[t] = x16

                    def transpose_group(t, g):
                        """4 PE transposes of feature chunks 4g..4g+3 of tile t
                        into one PSUM bank, drained by one ACT copy.  Tiles 0-1
                        transpose the raw fp32r directly (at startup there is
                        no score work to hide the fp16 downcast behind)."""
                        if g == 0:
                            hsts[t] = hstp.tile([P, E], F16, tag="hsT", name=f"hsT{t}")
                        hsT = hsts[t]
                        if t < 2:
                            pt = ptp.tile([P, 4 * P], F32R, tag="pt", name=f"pt{t}_{g}")
                            src_ap, idn = xts[t], ident
                        else:
                            pt = ptp.tile([P, 4 * P], F16, tag="pt", name=f"pt{t}_{g}")
                            src_ap, idn = x16s[t], ident_h
                        for u in range(4):
                            e = 4 * g + u
                            nc.tensor.transpose(
                                pt[:, u * P:(u + 1) * P],
                                src_ap[:, e * P:(e + 1) * P],
                                idn[:],
                            )
                        nc.scalar.copy(
                            out=hsT[:, g * 4 * P:(g + 1) * 4 * P],
                            in_=pt[:].bitcast(F32) if t < 2 else pt[:],
                        )

                    def score_chunk(t, g, sc_a, sc_b):
                        hsT = hsts[t]
                        sc_p = acc[:, t * H:(t + 1) * H]
                        for e in range(4 * g, 4 * g + 4):
                            lhs = hsT[:, e * P:(e + 1) * P]
                            nc.tensor.matmul(
                                sc_a[:], lhs, wsa[:, e, :],
                                start=(e == 0), stop=(e == EC - 1),
                            )
                            nc.tensor.matmul(
                                sc_b[:], lhs, wsb[:, e, :],
                                start=(e == 0), stop=(e == EC - 1),
                            )
                            nc.tensor.matmul(
                                sc_p, lhs, wsp[:, e, :],
                                start=(t == 0 and e == 0), stop=(e == EC - 1),
                            )

                    def combine(t, sc_a, sc_b):
                        mv, r, mr, msd = lns[t]
                        # base = sc_p * r - corr * (mu*r); reads the shared
                        # PSUM bank first so the next tile's plain matmuls
                        # never wait on this tile's combine.
                        t2 = cmb.tile([P, H], F32, tag="t2", name=f"t2_{t}")
                        nc.vector.tensor_scalar_mul(t2[:], corr[:, t, :], mr[:])
                        base = cmb.tile([P, H], F32, tag="base", name=f"base{t}")
                        nc.vector.scalar_tensor_tensor(
                            out=base[:], in0=acc[:, t * H:(t + 1) * H], scalar=r[:], in1=t2[:],
                            op0=mybir.AluOpType.mult, op1=mybir.AluOpType.subtract,
                        )
                        # cos/sin combine (DVE)
                        cb = ctab[:, t, :].unsqueeze(1).to_broadcast((P, H, R2))
                        sb = stab[:, t, :].unsqueeze(1).to_broadcast((P, H, R2))
                        m1 = cmbm.tile([P, H, R2], F32, tag="m1", name=f"m1_{t}")
                        m2 = cmbm.tile([P, H, R2], F32, tag="m2", name=f"m2_{t}")
                        nc.vector.tensor_tensor(
                            m1[:], sc_a[:].rearrange("p (h i) -> p h i", h=H), cb,
                            mybir.AluOpType.mult,
                        )
                        nc.vector.tensor_tensor(
                            m2[:], sc_b[:].rearrange("p (h i) -> p h i", h=H), sb,
                            mybir.AluOpType.mult,
                        )
                        m3 = cmbm.tile([P, H, R2], F32, tag="m3", name=f"m3_{t}")
                        nc.vector.tensor_tensor(m3[:], m1[:], m2[:], mybir.AluOpType.add)
                        red = cmb.tile([P, H], F32, tag="red", name=f"red{t}")
                        nc.vector.reduce_sum(
                            out=red[:], in_=m3[:], axis=mybir.AxisListType.X
                        )
                        sct = cmb.tile([P, H], F32, tag="sct", name=f"sct{t}")
                        nc.vector.scalar_tensor_tensor(
                            out=sct[:], in0=red[:], scalar=r[:], in1=base[:],
                            op0=mybir.AluOpType.mult, op1=mybir.AluOpType.add,
                        )
                        if has_corrb:
                            nc.vector.tensor_tensor(
                                sct[:], sct[:], corrb[:, t, :], mybir.AluOpType.add
                            )
                        es = cmb.tile([P, H], F32R, tag="es", name=f"es{t}")
                        nc.scalar.activation(
                            out=es[:], in_=sct[:],
                            func=mybir.ActivationFunctionType.Exp,
                        )
                        es2 = cmb.tile([P, H], F32R, tag="es2", name=f"es2_{t}")
                        nc.vector.tensor_scalar_mul(es2[:], es[:].bitcast(F32), r[:])
                        ess[t] = (es, es2)

                    def pool_accum(t):
                        """PE pooling matmuls for tile t (emitted one
                        iteration late so the DVE combine is long done)."""
                        xt = xts[t]
                        mv, r, mr, msd = lns[t]
                        es, es2 = ess[t]
                        for i in range(4):
                            nc.tensor.matmul(
                                z_ps[i][:], es2[:],
                                xt[:, i * 512:(i + 1) * 512],
                                start=(t == 0), stop=(t == TC - 1),
                            )
                        # [kappa | D] in one matmul: es = es2 * sd, so
                        # es2^T @ [mu | sd] = [sum es2*mu | sum es]
                        nc.tensor.matmul(
                            kd, es2[:], msd[:],
                            start=(t == 0), stop=(t == TC - 1),
                        )

                    # Software-pipelined emission.  PE queue order per tile t:
                    #   [T(t+1) group g | S(t) chunk g] x4, then Z(t-1).
                    # Transposes of t+1 fill the PSUM bank while the ACT copy
                    # of the previous group drains during the 12 score
                    # matmuls; pooling of t-1 runs while the DVE combine of t
                    # is still in flight.
                    # first two tiles go at the head of the SP DMA queue,
                    # ahead of the score weights
                    # one explicitly-ordered SP DMA stream: the FIFO
                    # gives hidden-state tiles priority over the tail-weight
                    # prefetch, and pool ring slots pace everything.
                    load(0)
                    nc.sync.dma_start(
                        wsa[:], in_wa.rearrange("(ec p) n -> p ec n", p=P))
                    load(1)
                    nc.sync.dma_start(
                        wsp[:], in_wp.rearrange("(ec p) n -> p ec n", p=P))
                    nc.sync.dma_start(
                        wsb[:], in_wb.rearrange("(ec p) n -> p ec n", p=P))
                    nc.sync.dma_start(
                        ctab[:], in_ct.rearrange("(t p) i -> p t i", p=P))
                    nc.sync.dma_start(
                        stab[:], in_st.rearrange("(t p) i -> p t i", p=P))
                    nc.sync.dma_start(
                        corr[:], in_co.rearrange("(t p) h -> p t h", p=P))
                    ln_stats(0)
                    for g in range(4):
                        transpose_group(0, g)
                    for t in range(TC):
                        if t + 2 < TC:
                            load(t + 2)
                        if t + 2 < TC:
                            convert16(t + 2)
                        sc_a = scp.tile([P, NA], F32, tag="sca", name=f"sca{t}")
                        sc_b = scp.tile([P, NB], F32, tag="scb", name=f"scb{t}")
                        for g in range(4):
                            if t + 1 < TC:
                                transpose_group(t + 1, g)
                            score_chunk(t, g, sc_a, sc_b)
                        if t >= 1:
                            pool_accum(t - 1)
                        combine(t, sc_a, sc_b)
                        if t + 1 < TC:
                            ln_stats(t + 1)
                        if t >= 7:
                            prefetch_tail_weight(t - 7)
                    pool_accum(TC - 1)
                    for k in range(TC - 7, 2 * NO):
                        prefetch_tail_weight(k)

                # ---------------- z normalization ----------------
                rd = sing.tile([H, 1], F32)
                nc.vector.reciprocal(out=rd[:], in_=kd[:, 1:2])
                kr = sing.tile([H, 1], F32)
                nc.vector.tensor_tensor(kr[:], kd[:, 0:1], rd[:], mybir.AluOpType.mult)
                z_sb = sing.tile([H, E], BF16)
                for i in range(4):
                    nc.vector.tensor_scalar(
                        out=z_sb[:, i * 512:(i + 1) * 512],
                        in0=z_ps[i][:], scalar1=rd[:], scalar2=kr[:],
                        op0=mybir.AluOpType.mult, op1=mybir.AluOpType.subtract,
                    )

            # ---------------- attn-out + out-projection ----------------
            with (
                tc.tile_pool(name="fin", bufs=1) as fin,
                tc.tile_pool(name="fps", bufs=1, space="PSUM") as fps,
            ):
                zT = fin.tile([P, EC, H], BF16)
                for i in range(EC):
                    pz = fps.tile([P, H], BF16, tag="pz", name=f"pz{i}")
                    nc.tensor.transpose(
                        pz[:], z_sb[:, i * P:(i + 1) * P], ident_bf[:H, :H]
                    )
                    nc.any.tensor_copy(out=zT[:, i, :], in_=pz[:])

                # attn-out: per head-block compute all 16 head columns
                # (N=16 keeps the matmul legal) then keep the block's own
                with tc.tile_pool(name="ops", bufs=3, space="PSUM") as ops:
                    oaT = fin.tile([P, EC], BF16)
                    vbT = None
                    if has_vbias:
                        vbT = fin.tile([P, EC], F32)
                        nc.sync.dma_start(vbT[:], in_vb[:])
                    for o in range(NO):
                        vwt = vw_tiles[o]
                        for hh in range(o * HPW, (o + 1) * HPW):
                            lo = (hh - o * HPW) * D
                            op = ops.tile([P, H], F32, tag="oa", name=f"oa{hh}")
                            for i in range(EC):
                                nc.tensor.matmul(
                                    op[:],
                                    vwt[:, i, lo:lo + D],
                                    zT[:, i, :],
                                    start=(i == 0), stop=(i == EC - 1),
                                )
                            if has_vbias:
                                nc.vector.tensor_tensor(
                                    oaT[:, hh:hh + 1], op[:, hh:hh + 1],
                                    vbT[:, hh:hh + 1], mybir.AluOpType.add,
                                )
                            else:
                                nc.vector.tensor_copy(
                                    out=oaT[:, hh:hh + 1], in_=op[:, hh:hh + 1]
                                )

                # out projection: final[o] = sum_e oaT[e] * owT[e, o];
                # each o-chunk is copied out and DMA'd as soon as it stops so
                # the single-partition drain overlaps the remaining matmuls.
                f_ps = fps.tile([1, E], F32, tag="fo")
                f_sb = fin.tile([1, E], F32)
                ob_t = None
                if has_obias:
                    ob_t = fin.tile([1, E], F32)
                    nc.sync.dma_start(ob_t[:], in_ob[:])
                for o in range(NO):
                    owt = ow_tiles[o]
                    for e in range(EC):
                        nc.tensor.matmul(
                            f_ps[:, o * NW:(o + 1) * NW],
                            oaT[:, e:e + 1],
                            owt[:, e, :],
                            start=(e == 0), stop=(e == EC - 1),
                        )
                    sl = slice(o * NW, (o + 1) * NW)
                    if has_obias:
                        nc.vector.tensor_tensor(
                            f_sb[:, sl], f_ps[:, sl], ob_t[:, sl],
                            mybir.AluOpType.add,
                        )
                    else:
                        nc.vector.tensor_copy(out=f_sb[:, sl], in_=f_ps[:, sl])
                    nc.sync.dma_start(out_t[:, sl], f_sb[:, sl])

    nc.compile()
    return nc


def _prep_host(inputs):
    hs = np.ascontiguousarray(np.asarray(inputs["hidden_states"], dtype=np.float32))
    ids = np.asarray(inputs["input_ids_with_pads"])
    ln_w = np.asarray(inputs["ln_w"], dtype=np.float64)
    ln_b = np.asarray(inputs["ln_b"], dtype=np.float64)
    k_w = np.asarray(inputs["k_w"], dtype=np.float64)
    q_w = np.asarray(inputs["q_w"], dtype=np.float64)
    v_w = np.asarray(inputs["v_w"], dtype=np.float32)
    o_w = np.asarray(inputs["out_w"], dtype=np.float32)
    k_b = np.asarray(inputs["k_b"], dtype=np.float64)
    q_b = np.asarray(inputs["q_b"], dtype=np.float64)
    v_b = np.asarray(inputs["v_b"], dtype=np.float64)
    o_b = np.asarray(inputs["out_b"], dtype=np.float32)

    # last non-pad token index per row
    ix = np.argmax(np.cumsum((ids != PAD).astype(np.int64), axis=1), axis=1)

    # exact q per batch row (host-side: tiny)
    hl = hs[np.arange(B), ix].astype(np.float64)            # [B,E]
    mu = hl.mean(-1, keepdims=True)
    var = ((hl - mu) ** 2).mean(-1, keepdims=True)
    hln = (hl - mu) / np.sqrt(var + EPS) * ln_w + ln_b
    q = hln @ q_w.T + q_b                                    # [B,E]

    # rotary tables
    inv = 1.0 / (10000.0 ** (np.arange(0, ROT, 2, dtype=np.float64) / ROT))
    ang = np.arange(S, dtype=np.float64)[:, None] * inv[None, :]
    cosd, sind = np.cos(ang), np.sin(ang)                    # [S,R2]

    # fused score-projection per batch: W_sc = [A | B | plain]
    Wk3 = (k_w * ln_w[None, :]).reshape(H, D, E)             # [H,D,E]
    Wp = Wk3[:, :ROT].reshape(H, R2, 2, E)
    q3 = q.reshape(B, H, D)
    qp = q3[:, :, :ROT].reshape(B, H, R2, 2)
    A = qp[..., 0, None] * Wp[None, :, :, 0] + qp[..., 1, None] * Wp[None, :, :, 1]
    Bm = qp[..., 1, None] * Wp[None, :, :, 0] - qp[..., 0, None] * Wp[None, :, :, 1]
    Pl = np.einsum('bhd,hde->bhe', q3[:, :, ROT:], Wk3[:, ROT:])   # [B,H,E]
    sA, sB, sP = A.sum(-1), Bm.sum(-1), Pl.sum(-1)
    corr = (np.einsum('si,bhi->bsh', cosd, sA)
            + np.einsum('si,bhi->bsh', sind, sB) + sP[:, None, :])  # [B,S,H]

    # k-bias contribution (constant per (t,h) score offset)
    kb_eff = (ln_b @ k_w.T + k_b).reshape(H, D)
    kbp = kb_eff[:, :ROT].reshape(H, R2, 2)
    biasA = qp[..., 0] * kbp[None, ..., 0] + qp[..., 1] * kbp[None, ..., 1]
    biasB = qp[..., 1] * kbp[None, ..., 0] - qp[..., 0] * kbp[None, ..., 1]
    biasP = np.einsum('bhd,hd->bh', q3[:, :, ROT:], kb_eff[:, ROT:])
    corrb = (np.einsum('si,bhi->bsh', cosd, biasA)
             + np.einsum('si,bhi->bsh', sind, biasB) + biasP[:, None, :])

    vwT = np.ascontiguousarray(
        (v_w * np.float32(1) * ln_w[None, :].astype(np.float32)).T
        .astype(ml_dtypes.bfloat16))
    owT = np.ascontiguousarray(o_w.T.astype(ml_dtypes.bfloat16))
    vbias = (ln_b @ v_w.T.astype(np.float64) + v_b)
    obias = o_b

    flags = (
        bool(np.any(corrb)), bool(np.any(vbias)), bool(np.any(obias)),
    )

    shared = {
        "vwT": vwT, "owT": owT,
        "ctab": np.ascontiguousarray(cosd.astype(np.float32)),
        "stab": np.ascontiguousarray(sind.astype(np.float32)),
    }
    if flags[1]:
        shared["vbiasT"] = np.ascontiguousarray(
            vbias.astype(np.float32).reshape(EC, P).T)
    if flags[2]:
        shared["obias"] = np.ascontiguousarray(obias[None, :])

    in_maps = []
    for b in range(B):
        m = dict(shared)
        m["hs"] = np.ascontiguousarray(hs[b])
        m["wsa"] = np.ascontiguousarray(
            A[b].transpose(2, 0, 1).reshape(E, NA).astype(np.float16))
        m["wsb"] = np.ascontiguousarray(
            Bm[b].transpose(2, 0, 1).reshape(E, NB).astype(np.float16))
        m["wsp"] = np.ascontiguousarray(Pl[b].T.astype(np.float16))
        m["corr"] = np.ascontiguousarray(corr[b].astype(np.float32))
        if flags[0]:
            m["corrb"] = np.ascontiguousarray(corrb[b].astype(np.float32))
        in_maps.append(m)
    return flags, in_maps


def kernel(**inputs):
    flags, in_maps = _prep_host(inputs)
    if flags not in _CACHE:
        _CACHE[flags] = _build_program(flags)
    nc = _CACHE[flags]
    res = bass_utils.run_bass_kernel_spmd(nc, in_maps, core_ids=list(range(B)))
    out = np.stack([res.results[b]["out"][0] for b in range(B)], axis=0)
    return out.astype(np.float32)


# revision 48
# speedup vs baseline: 1.0264x; 1.0264x over previous
"""NostARHead attention kernel for Trainium2 (8 NeuronCores, batch-parallel).

Strategy
--------
Data-parallel over batch: core b handles batch element b (B == n_cores == 8).

Algebraic structure exploited:
  1. The single query token sits at rotary position 0, where the RoPE
     rotation is the identity (sin 0 = 0, cos 0 = 1).  The attention score
     therefore factors through a fixed basis of cos/sin position features:
        score[t,h] = sum_j cos(th_j t) * a[t,h,j] + sin(th_j t) * b[t,h,j]
                     + plain[t,h]
     where a, b, plain are LINEAR in hs[t] with weights combining q and W_k.
     Since the last-token index (and hence q) is host-computable, the host
     builds a fused score-projection matrix W_sc = [A | B | plain] of shape
     [E, 2*H*R2 + H] = [2048, 1040] -- roughly HALF the FLOPs of the full
     K-projection, with no on-device RoPE and no on-device q-path at all.
     W_sc is fp16 (10-bit mantissa; scores accumulate in fp32 PSUM).
  2. LayerNorm's per-token affine commutes with everything downstream:
     the device transposes and pools the RAW hidden states and applies the
     (r_t, mu_t) correction to the 16 scores per token (score path) and as
     a rank-1 correction kappa to the pooled vector (value path):
        score[t]  = r_t * combo_raw[t] - (r_t mu_t) * corr_s[t]   (+corr_b)
        z[h]      = (sum_t es[t,h] r_t hs_raw[t] - kappa[h] 1) / D_h
     corr_s is a host-precomputed [S,H] table.  r_t = rsqrt(var+eps) is a
     3-step Newton iteration on GPSIMD (var is within a few percent of 1),
     keeping the ACT engine exp-only (no activation-table reloads) and the
     serial chain off the DVE queue.
  3. With a single query the V-projection commutes with attention pooling:
     pool first (z = es^T @ hs_raw, an fp32r [H,E] matmul against the raw
     tile), then apply W_v per head (bf16), then the out-projection (bf16).
     kappa and the softmax denominator come from one extra matmul
     es2^T @ [mu | sd] accumulated in a shared PSUM bank whose has_written
     bits are never cleared after t=0 (plain regions are per-tile columns).

Scheduling
----------
Per 128-token tile, software-pipelined: PE runs [transpose-group(t+1) |
score-chunk(t)] x4 then the pooling matmuls of t-1 (so it never waits on
the DVE combine); ACT drains transpose PSUM and runs exp; DVE runs
bn_stats and the cos/sin combine; GPSIMD runs the rsqrt chain.  All DMA is
issued on one explicitly-ordered SP stream (hidden-state tiles ahead of
the paced vw/ow prefetch; pool ring slots provide flow control).  Tiles
t >= 2 are downcast to fp16 on ACT so their transposes run at 1 cycle/row.
The output is copied and DMA'd per 256-column chunk as the out-projection
completes.

The module compiles the program once (shapes are static) and caches it.
"""

import numpy as np
import ml_dtypes

import concourse.bass as bass
import concourse.mybir as mybir
import concourse.tile as tile
from concourse import bacc, bass_utils
from concourse.masks import make_identity

F32 = mybir.dt.float32
F32R = mybir.dt.float32r
F16 = mybir.dt.float16
BF16 = mybir.dt.bfloat16

P = 128
B = 8
S = 2048
E = 2048
H = 16
D = 128
ROT = 64
R2 = ROT // 2        # 32 rotary pairs per head
PAD = 50257
EPS = 1e-5

EC = E // P          # 16 feature chunks
TC = S // P          # 16 token tiles
NA = H * R2          # 512 cols in the A (cos) block
NB = H * R2          # 512 cols in the B (sin) block
NW = 256             # weight free-dim slice for the tail projections
NO = E // NW         # 8 output-dim slices
HPW = NW // D        # heads per weight slice (2)

_CACHE = {}


def _build_program(flags):
    """Per-core SPMD program. flags: (has_corrb, has_vbias, has_obias)"""
    has_corrb, has_vbias, has_obias = flags
    nc = bacc.Bacc("TRN2", debug=False, num_devices=B)

    in_hs = nc.dram_tensor("hs", [S, E], F32R, kind="ExternalInput").ap()
    in_wa = nc.dram_tensor("wsa", [E, NA], F16, kind="ExternalInput").ap()
    in_wb = nc.dram_tensor("wsb", [E, NB], F16, kind="ExternalInput").ap()
    in_wp = nc.dram_tensor("wsp", [E, H], F16, kind="ExternalInput").ap()
    in_vw = nc.dram_tensor("vwT", [E, E], BF16, kind="ExternalInput").ap()
    in_ow = nc.dram_tensor("owT", [E, E], BF16, kind="ExternalInput").ap()
    in_ct = nc.dram_tensor("ctab", [S, R2], F32, kind="ExternalInput").ap()
    in_st = nc.dram_tensor("stab", [S, R2], F32, kind="ExternalInput").ap()
    in_co = nc.dram_tensor("corr", [S, H], F32, kind="ExternalInput").ap()
    in_cb = in_vb = in_ob = None
    if has_corrb:
        in_cb = nc.dram_tensor("corrb", [S, H], F32, kind="ExternalInput").ap()
    if has_vbias:
        in_vb = nc.dram_tensor("vbiasT", [P, EC], F32, kind="ExternalInput").ap()
    if has_obias:
        in_ob = nc.dram_tensor("obias", [1, E], F32, kind="ExternalInput").ap()
    out_t = nc.dram_tensor("out", [1, E], F32, kind="ExternalOutput").ap()

    with tile.TileContext(nc) as tc:
        with (
            tc.tile_pool(name="sing", bufs=1) as sing,
            tc.tile_pool(name="wvp", bufs=12) as wvp,
        ):
            # ---------------- constants / preloads ----------------
            ident32 = sing.tile([P, P], F32)
            make_identity(nc, ident32[:])
            ident = sing.tile([P, P], F32R)
            nc.vector.tensor_copy(out=ident[:], in_=ident32[:])
            ident_bf = sing.tile([P, P], BF16)
            nc.vector.tensor_copy(out=ident_bf[:], in_=ident32[:])
            ident_h = sing.tile([P, P], F16)
            nc.vector.tensor_copy(out=ident_h[:], in_=ident32[:])
            ctab = sing.tile([P, TC, R2], F32)
            stab = sing.tile([P, TC, R2], F32)
            corr = sing.tile([P, TC, H], F32)
            corrb = None
            if has_corrb:
                corrb = sing.tile([P, TC, H], F32)
                nc.sync.dma_start(corrb[:], in_cb.rearrange("(t p) h -> p t h", p=P))
            # fused score-projection weights, fully resident
            wsa = sing.tile([P, EC, NA], F16)
            wsb = sing.tile([P, EC, NB], F16)
            wsp = sing.tile([P, EC, H], F16)
            # prefetch the value/out projection weights; DMAs are queued
            # behind the score weights and stream in during the main loop.
            vw_tiles = []
            ow_tiles = []

            def prefetch_tail_weight(k):
                """One vw/ow chunk DMA, paced from inside the main loop so the
                prefetch never starves the hidden-state stream."""
                if k < NO:
                    vwt = wvp.tile([P, EC, NW], BF16, tag="wv", name=f"vw{k}")
                    nc.sync.dma_start(
                        vwt[:],
                        in_vw.rearrange("(ec p) o -> p ec o", p=P)[
                            :, :, k * NW:(k + 1) * NW
                        ],
                    )
                    vw_tiles.append(vwt)
                elif k < 2 * NO:
                    o = k - NO
                    owt = wvp.tile([P, EC, NW], BF16, tag="wv", name=f"ow{o}")
                    nc.sync.dma_start(
                        owt[:],
                        in_ow.rearrange("(ec p) o -> p ec o", p=P)[
                            :, :, o * NW:(o + 1) * NW
                        ],
                    )
                    ow_tiles.append(owt)

            # ---------------- main loop ----------------
            with tc.tile_pool(name="zps", bufs=1, space="PSUM") as zps:
                z_ps = [
                    zps.tile([H, 512], F32, tag=f"z{i}", name=f"z{i}")
                    for i in range(4)
                ]
                # one shared bank: 16 per-tile plain regions + [kappa|D].
                # Only two matmuls ever carry start=True in this bank (plain
                # t=0 and kd t=0), so the bank-wide has_written clear cannot
                # wipe the persistent kd accumulation.
                acc = zps.tile([P, 512], F32, tag="acc", name="acc")
                kd = acc[:H, 256:258]       # [kappa | D] accumulators (persistent)

                with (
                    tc.tile_pool(name="xtp", bufs=4) as xtp,
                    tc.tile_pool(name="lnp", bufs=3) as lnp,
                    tc.tile_pool(name="hstp", bufs=2) as hstp,
                    tc.tile_pool(name="x16p", bufs=3) as x16p,
                    tc.tile_pool(name="cmb", bufs=3) as cmb,
                    tc.tile_pool(name="cmbm", bufs=2) as cmbm,
                    tc.tile_pool(name="ptp", bufs=1, space="PSUM") as ptp,
                    tc.tile_pool(name="scp", bufs=1, space="PSUM") as scp,
                ):
                    xts = [None] * TC
                    x16s = [None] * TC
                    lns = [None] * TC
                    hsts = [None] * TC
                    ess = [None] * TC

                    def load(t):
                        xt = xtp.tile([P, E], F32R, tag="xt", name=f"xt{t}")
                        nc.sync.dma_start(xt[:], in_hs[t * P:(t + 1) * P, :])
                        xts[t] = xt

                    def ln_stats(t):
                        xt = xts[t]
                        stats = lnp.tile([P, 4, 6], F32, tag="st", name=f"st{t}")
                        for g in range(4):
                            nc.vector.bn_stats(
                                out=stats[:, g, :], in_=xt[:, g * 512:(g + 1) * 512].bitcast(F32)
                            )
                        mv = lnp.tile([P, 2], F32, tag="mv", name=f"mv{t}")
                        nc.vector.bn_aggr(out=mv[:], in_=stats[:])
                        # r = rsqrt(var + EPS) via Newton on DVE (keeps the
                        # ACT engine exp-only, avoiding act-table reloads).
                        # var is within a few percent of 1, so y0 = 1 and
                        # three iterations reach float32 roundoff.
                        # The whole serial rsqrt chain runs on the
                        # otherwise-idle GPSIMD engine: a dependent chain of
                        # tiny ops would convoy the DVE's in-order queue and
                        # stall the score combine behind it.
                        var = mv[:, 1:2]
                        vpe = lnp.tile([P, 1], F32, tag="vpe", name=f"vpe{t}")
                        r = lnp.tile([P, 1], F32, tag="r", name=f"r{t}")
                        tq = lnp.tile([P, 1], F32, tag="tq", name=f"tq{t}")
                        uq = lnp.tile([P, 1], F32, tag="uq", name=f"uq{t}")
                        nc.gpsimd.tensor_scalar_add(vpe[:], var, float(EPS))
                        nc.gpsimd.tensor_scalar_mul(r[:], vpe[:], -0.5)
                        nc.gpsimd.tensor_scalar_add(r[:], r[:], 1.5)
                        for _ in range(2):
                            nc.gpsimd.tensor_tensor(
                                tq[:], r[:], r[:], mybir.AluOpType.mult
                            )
                            nc.gpsimd.tensor_tensor(
                                uq[:], vpe[:], tq[:], mybir.AluOpType.mult
                            )
                            nc.gpsimd.tensor_scalar_mul(uq[:], uq[:], -0.5)
                            nc.gpsimd.tensor_scalar_add(uq[:], uq[:], 1.5)
                            nc.gpsimd.tensor_tensor(
                                r[:], r[:], uq[:], mybir.AluOpType.mult
                            )
                        mr = lnp.tile([P, 1], F32, tag="mr", name=f"mr{t}")
                        nc.gpsimd.tensor_tensor(
                            mr[:], mv[:, 0:1], r[:], mybir.AluOpType.mult
                        )
                        # msd = [mu | sd];  sd = (var+EPS) * r  (DVE: its F32R
                        # write must come from a rounding engine op)
                        msd = lnp.tile([P, 2], F32R, tag="msd", name=f"msd{t}")
                        nc.vector.tensor_copy(out=msd[:, 0:1], in_=mv[:, 0:1])
                        nc.vector.tensor_tensor(
                            msd[:, 1:2], vpe[:], r[:], mybir.AluOpType.mult
                        )
                        lns[t] = (mv, r, mr, msd)

                    def convert16(t):
                        """Downcast tile t to fp16 (ACT) so its transposes run
                        at 1 cycle/row instead of fp32r's 1.5."""
                        x16 = x16p.tile([P, E], F16, tag="x16", name=f"x16_{t}")
                        nc.scalar.copy(out=x16[:], in_=xts[t][:].bitcast(F32))
                        x16s[t] = x16

                    def transpose_group(t, g):
                        """4 PE transposes of feature chunks 4g..4g+3 of tile t
                        into one PSUM bank, drained by one ACT copy.  Tiles 0-1
                        transpose the raw fp32r directly (at startup there is
                        no score work to hide the fp16 downcast behind)."""
                        if g == 0:
                            hsts[t] = hstp.tile([P, E], F16, tag="hsT", name=f"hsT{t}")
                        hsT = hsts[t]
                        if t < 2:
                            pt = ptp.tile([P, 4 * P], F32R, tag="pt", name=f"pt{t}_{g}")
                            src_ap, idn = xts[t], ident
                        else:
                            pt = ptp.tile([P, 4 * P], F16, tag="pt", name=f"pt{t}_{g}")
                            src_ap, idn = x16s[t], ident_h
                        for u in range(4):
                            e = 4 * g + u
                            nc.tensor.transpose(
                                pt[:, u * P:(u + 1) * P],
                                src_ap[:, e * P:(e + 1) * P],
                                idn[:],
                            )
                        nc.scalar.copy(
                            out=hsT[:, g * 4 * P:(g + 1) * 4 * P],
                            in_=pt[:].bitcast(F32) if t < 2 else pt[:],
                        )

                    def score_chunk(t, g, sc_a, sc_b):
                        hsT = hsts[t]
                        sc_p = acc[:, t * H:(t + 1) * H]
                        for e in range(4 * g, 4 * g + 4):
                            lhs = hsT[:, e * P:(e + 1) * P]
                            nc.tensor.matmul(
                                sc_a[:], lhs, wsa[:, e, :],
                                start=(e == 0), stop=(e == EC - 1),
                            )
                            nc.tensor.matmul(
                                sc_b[:], lhs, wsb[:, e, :],
                                start=(e == 0), stop=(e == EC - 1),
                            )
                            nc.tensor.matmul(
                                sc_p, lhs, wsp[:, e, :],
                                start=(t == 0 and e == 0), stop=(e == EC - 1),
                            )

                    def combine(t, sc_a, sc_b):
                        mv, r, mr, msd = lns[t]
                        # base = sc_p * r - corr * (mu*r); reads the shared
                        # PSUM bank first so the next tile's plain matmuls
                        # never wait on this tile's combine.
                        t2 = cmb.tile([P, H], F32, tag="t2", name=f"t2_{t}")
                        nc.vector.tensor_scalar_mul(t2[:], corr[:, t, :], mr[:])
                        base = cmb.tile([P, H], F32, tag="base", name=f"base{t}")
                        nc.vector.scalar_tensor_tensor(
                            out=base[:], in0=acc[:, t * H:(t + 1) * H], scalar=r[:], in1=t2[:],
                            op0=mybir.AluOpType.mult, op1=mybir.AluOpType.subtract,
                        )
                        # cos/sin combine (DVE)
                        cb = ctab[:, t, :].unsqueeze(1).to_broadcast((P, H, R2))
                        sb = stab[:, t, :].unsqueeze(1).to_broadcast((P, H, R2))
                        m1 = cmbm.tile([P, H, R2], F32, tag="m1", name=f"m1_{t}")
                        m2 = cmbm.tile([P, H, R2], F32, tag="m2", name=f"m2_{t}")
                        nc.vector.tensor_tensor(
                            m1[:], sc_a[:].rearrange("p (h i) -> p h i", h=H), cb,
                            mybir.AluOpType.mult,
                        )
                        nc.vector.tensor_tensor(
                            m2[:], sc_b[:].rearrange("p (h i) -> p h i", h=H), sb,
                            mybir.AluOpType.mult,
                        )
                        m3 = cmbm.tile([P, H, R2], F32, tag="m3", name=f"m3_{t}")
                        nc.vector.tensor_tensor(m3[:], m1[:], m2[:], mybir.AluOpType.add)
                        red = cmb.tile([P, H], F32, tag="red", name=f"red{t}")
                        nc.vector.reduce_sum(
                            out=red[:], in_=m3[:], axis=mybir.AxisListType.X
                        )
                        sct = cmb.tile([P, H], F32, tag="sct", name=f"sct{t}")
                        nc.vector.scalar_tensor_tensor(
                            out=sct[:], in0=red[:], scalar=r[:], in1=base[:],
                            op0=mybir.AluOpType.mult, op1=mybir.AluOpType.add,
                        )
                        if has_corrb:
                            nc.vector.tensor_tensor(
                                sct[:], sct[:], corrb[:, t, :], mybir.AluOpType.add
                            )
                        es = cmb.tile([P, H], F32R, tag="es", name=f"es{t}")
                        nc.scalar.activation(
                            out=es[:], in_=sct[:],
                            func=mybir.ActivationFunctionType.Exp,
                        )
                        es2 = cmb.tile([P, H], F32R, tag="es2", name=f"es2_{t}")
                        nc.vector.tensor_scalar_mul(es2[:], es[:].bitcast(F32), r[:])
                        ess[t] = (es, es2)

                    def pool_accum(t):
                        """PE pooling matmuls for tile t (emitted one
                        iteration late so the DVE combine is long done)."""
                        xt = xts[t]
                        mv, r, mr, msd = lns[t]
                        es, es2 = ess[t]
                        for i in range(4):
                            nc.tensor.matmul(
                                z_ps[i][:], es2[:],
                                xt[:, i * 512:(i + 1) * 512],
                                start=(t == 0), stop=(t == TC - 1),
                            )
                        # [kappa | D] in one matmul: es = es2 * sd, so
                        # es2^T @ [mu | sd] = [sum es2*mu | sum es]
                        nc.tensor.matmul(
                            kd, es2[:], msd[:],
                            start=(t == 0), stop=(t == TC - 1),
                        )

                    # Software-pipelined emission.  PE queue order per tile t:
                    #   [T(t+1) group g | S(t) chunk g] x4, then Z(t-1).
                    # Transposes of t+1 fill the PSUM bank while the ACT copy
                    # of the previous group drains during the 12 score
                    # matmuls; pooling of t-1 runs while the DVE combine of t
                    # is still in flight.
                    # first two tiles go at the head of the SP DMA queue,
                    # ahead of the score weights
                    # one explicitly-ordered SP DMA stream: the FIFO
                    # gives hidden-state tiles priority over the tail-weight
                    # prefetch, and pool ring slots pace everything.
                    load(0)
                    nc.sync.dma_start(
                        wsa[:], in_wa.rearrange("(ec p) n -> p ec n", p=P))
                    load(1)
                    nc.sync.dma_start(
                        wsp[:], in_wp.rearrange("(ec p) n -> p ec n", p=P))
                    nc.sync.dma_start(
                        wsb[:], in_wb.rearrange("(ec p) n -> p ec n", p=P))
                    nc.sync.dma_start(
                        ctab[:], in_ct.rearrange("(t p) i -> p t i", p=P))
                    nc.sync.dma_start(
                        stab[:], in_st.rearrange("(t p) i -> p t i", p=P))
                    nc.sync.dma_start(
                        corr[:], in_co.rearrange("(t p) h -> p t h", p=P))
                    ln_stats(0)
                    for g in range(4):
                        transpose_group(0, g)
                    for t in range(TC):
                        if t + 2 < TC:
                            load(t + 2)
                        if t + 2 < TC:
                            convert16(t + 2)
                        sc_a = scp.tile([P, NA], F32, tag="sca", name=f"sca{t}")
                        sc_b = scp.tile([P, NB], F32, tag="scb", name=f"scb{t}")
                        for g in range(4):
                            if t + 1 < TC:
                                transpose_group(t + 1, g)
                            score_chunk(t, g, sc_a, sc_b)
                        if t >= 1:
                            pool_accum(t - 1)
                        combine(t, sc_a, sc_b)
                        if t + 1 < TC:
                            ln_stats(t + 1)
                        if t >= 7:
                            prefetch_tail_weight(t - 7)
                    pool_accum(TC - 1)
                    for k in range(TC - 7, 2 * NO):
                        prefetch_tail_weight(k)

                # ---------------- z normalization ----------------
                rd = sing.tile([H, 1], F32)
                nc.vector.reciprocal(out=rd[:], in_=kd[:, 1:2])
                z_sb = sing.tile([H, E], BF16)
                for i in range(4):
                    nc.vector.tensor_scalar(
                        out=z_sb[:, i * 512:(i + 1) * 512],
                        in0=z_ps[i][:], scalar1=kd[:, 0:1], scalar2=rd[:],
                        op0=mybir.AluOpType.subtract, op1=mybir.AluOpType.mult,
                    )

            # ---------------- attn-out + out-projection ----------------
            with (
                tc.tile_pool(name="fin", bufs=1) as fin,
                tc.tile_pool(name="pzp", bufs=2, space="PSUM") as pzp,
                tc.tile_pool(name="fps", bufs=1, space="PSUM") as fps,
            ):
                zT = fin.tile([P, EC, H], BF16)
                for i in range(EC):
                    pz = pzp.tile([P, H], BF16, tag="pz", name=f"pz{i}")
                    nc.tensor.transpose(
                        pz[:], z_sb[:, i * P:(i + 1) * P], ident_bf[:H, :H]
                    )
                    nc.any.tensor_copy(out=zT[:, i, :], in_=pz[:])

                # attn-out: per head-block compute all 16 head columns
                # (N=16 keeps the matmul legal) then keep the block's own
                with tc.tile_pool(name="ops", bufs=2, space="PSUM") as ops:
                    oaT = fin.tile([P, EC], BF16)
                    vbT = None
                    if has_vbias:
                        vbT = fin.tile([P, EC], F32)
                        nc.sync.dma_start(vbT[:], in_vb[:])
                    for o in range(NO):
                        vwt = vw_tiles[o]
                        for hh in range(o * HPW, (o + 1) * HPW):
                            lo = (hh - o * HPW) * D
                            op = ops.tile([P, H], F32, tag="oa", name=f"oa{hh}")
                            for i in range(EC):
                                nc.tensor.matmul(
                                    op[:],
                                    vwt[:, i, lo:lo + D],
                                    zT[:, i, :],
                                    start=(i == 0), stop=(i == EC - 1),
                                )
                            if has_vbias:
                                nc.vector.tensor_tensor(
                                    oaT[:, hh:hh + 1], op[:, hh:hh + 1],
                                    vbT[:, hh:hh + 1], mybir.AluOpType.add,
                                )
                            else:
                                nc.vector.tensor_copy(
                                    out=oaT[:, hh:hh + 1], in_=op[:, hh:hh + 1]
                                )

                # out projection: final[o] = sum_e oaT[e] * owT[e, o];
                # each o-chunk is copied out and DMA'd as soon as it stops so
                # the single-partition drain overlaps the remaining matmuls.
                f_ps = fps.tile([1, E], F32, tag="fo")
                f_sb = fin.tile([1, E], F32)
                ob_t = None
                if has_obias:
                    ob_t = fin.tile([1, E], F32)
                    nc.sync.dma_start(ob_t[:], in_ob[:])
                for o in range(NO):
                    owt = ow_tiles[o]
                    for e in range(EC):
                        nc.tensor.matmul(
                            f_ps[:, o * NW:(o + 1) * NW],
                            oaT[:, e:e + 1],
                            owt[:, e, :],
                            start=(e == 0), stop=(e == EC - 1),
                        )
                    sl = slice(o * NW, (o + 1) * NW)
                    if has_obias:
                        nc.vector.tensor_tensor(
                            f_sb[:, sl], f_ps[:, sl], ob_t[:, sl],
                            mybir.AluOpType.add,
                        )
                    else:
                        nc.vector.tensor_copy(out=f_sb[:, sl], in_=f_ps[:, sl])
                    nc.sync.dma_start(out_t[:, sl], f_sb[:, sl])

    nc.compile()
    return nc


def _prep_host(inputs):
    hs = np.ascontiguousarray(np.asarray(inputs["hidden_states"], dtype=np.float32))
    ids = np.asarray(inputs["input_ids_with_pads"])
    ln_w = np.asarray(inputs["ln_w"], dtype=np.float64)
    ln_b = np.asarray(inputs["ln_b"], dtype=np.float64)
    k_w = np.asarray(inputs["k_w"], dtype=np.float64)
    q_w = np.asarray(inputs["q_w"], dtype=np.float64)
    v_w = np.asarray(inputs["v_w"], dtype=np.float32)
    o_w = np.asarray(inputs["out_w"], dtype=np.float32)
    k_b = np.asarray(inputs["k_b"], dtype=np.float64)
    q_b = np.asarray(inputs["q_b"], dtype=np.float64)
    v_b = np.asarray(inputs["v_b"], dtype=np.float64)
    o_b = np.asarray(inputs["out_b"], dtype=np.float32)

    # last non-pad token index per row
    ix = np.argmax(np.cumsum((ids != PAD).astype(np.int64), axis=1), axis=1)

    # exact q per batch row (host-side: tiny)
    hl = hs[np.arange(B), ix].astype(np.float64)            # [B,E]
    mu = hl.mean(-1, keepdims=True)
    var = ((hl - mu) ** 2).mean(-1, keepdims=True)
    hln = (hl - mu) / np.sqrt(var + EPS) * ln_w + ln_b
    q = hln @ q_w.T + q_b                                    # [B,E]

    # rotary tables
    inv = 1.0 / (10000.0 ** (np.arange(0, ROT, 2, dtype=np.float64) / ROT))
    ang = np.arange(S, dtype=np.float64)[:, None] * inv[None, :]
    cosd, sind = np.cos(ang), np.sin(ang)                    # [S,R2]

    # fused score-projection per batch: W_sc = [A | B | plain]
    Wk3 = (k_w * ln_w[None, :]).reshape(H, D, E)             # [H,D,E]
    Wp = Wk3[:, :ROT].reshape(H, R2, 2, E)
    q3 = q.reshape(B, H, D)
    qp = q3[:, :, :ROT].reshape(B, H, R2, 2)
    A = qp[..., 0, None] * Wp[None, :, :, 0] + qp[..., 1, None] * Wp[None, :, :, 1]
    Bm = qp[..., 1, None] * Wp[None, :, :, 0] - qp[..., 0, None] * Wp[None, :, :, 1]
    Pl = np.einsum('bhd,hde->bhe', q3[:, :, ROT:], Wk3[:, ROT:])   # [B,H,E]
    sA, sB, sP = A.sum(-1), Bm.sum(-1), Pl.sum(-1)
    corr = (np.einsum('si,bhi->bsh', cosd, sA)
            + np.einsum('si,bhi->bsh', sind, sB) + sP[:, None, :])  # [B,S,H]

    # k-bias contribution (constant per (t,h) score offset)
    kb_eff = (ln_b @ k_w.T + k_b).reshape(H, D)
    kbp = kb_eff[:, :ROT].reshape(H, R2, 2)
    biasA = qp[..., 0] * kbp[None, ..., 0] + qp[..., 1] * kbp[None, ..., 1]
    biasB = qp[..., 1] * kbp[None, ..., 0] - qp[..., 0] * kbp[None, ..., 1]
    biasP = np.einsum('bhd,hd->bh', q3[:, :, ROT:], kb_eff[:, ROT:])
    corrb = (np.einsum('si,bhi->bsh', cosd, biasA)
             + np.einsum('si,bhi->bsh', sind, biasB) + biasP[:, None, :])

    vwT = np.ascontiguousarray(
        (v_w * np.float32(1) * ln_w[None, :].astype(np.float32)).T
        .astype(ml_dtypes.bfloat16))
    owT = np.ascontiguousarray(o_w.T.astype(ml_dtypes.bfloat16))
    vbias = (ln_b @ v_w.T.astype(np.float64) + v_b)
    obias = o_b

    flags = (
        bool(np.any(corrb)), bool(np.any(vbias)), bool(np.any(obias)),
    )

    shared = {
        "vwT": vwT, "owT": owT,
        "ctab": np.ascontiguousarray(cosd.astype(np.float32)),
        "stab": np.ascontiguousarray(sind.astype(np.float32)),
    }
    if flags[1]:
        shared["vbiasT"] = np.ascontiguousarray(
            vbias.astype(np.float32).reshape(EC, P).T)
    if flags[2]:
        shared["obias"] = np.ascontiguousarray(obias[None, :])

    in_maps = []
    for b in range(B):
        m = dict(shared)
        m["hs"] = np.ascontiguousarray(hs[b])
        m["wsa"] = np.ascontiguousarray(
            A[b].transpose(2, 0, 1).reshape(E, NA).astype(np.float16))
        m["wsb"] = np.ascontiguousarray(
            Bm[b].transpose(2, 0, 1).reshape(E, NB).astype(np.float16))
        m["wsp"] = np.ascontiguousarray(Pl[b].T.astype(np.float16))
        m["corr"] = np.ascontiguousarray(corr[b].astype(np.float32))
        if flags[0]:
            m["corrb"] = np.ascontiguousarray(corrb[b].astype(np.float32))
        in_maps.append(m)
    return flags, in_maps


def kernel(**inputs):
    flags, in_maps = _prep_host(inputs)
    if flags not in _CACHE:
        _CACHE[flags] = _build_program(flags)
    nc = _CACHE[flags]
    res = bass_utils.run_bass_kernel_spmd(nc, in_maps, core_ids=list(range(B)))
    out = np.stack([res.results[b]["out"][0] for b in range(B)], axis=0)
    return out.astype(np.float32)


# revision 49
# speedup vs baseline: 1.0350x; 1.0084x over previous
"""NostARHead attention kernel for Trainium2 (8 NeuronCores, batch-parallel).

Strategy
--------
Data-parallel over batch: core b handles batch element b (B == n_cores == 8).

Algebraic structure exploited:
  1. The single query token sits at rotary position 0, where the RoPE
     rotation is the identity (sin 0 = 0, cos 0 = 1).  The attention score
     therefore factors through a fixed basis of cos/sin position features:
        score[t,h] = sum_j cos(th_j t) * a[t,h,j] + sin(th_j t) * b[t,h,j]
                     + plain[t,h]
     where a, b, plain are LINEAR in hs[t] with weights combining q and W_k.
     Since the last-token index (and hence q) is host-computable, the host
     builds a fused score-projection matrix W_sc = [A | B | plain] of shape
     [E, 2*H*R2 + H] = [2048, 1040] -- roughly HALF the FLOPs of the full
     K-projection, with no on-device RoPE and no on-device q-path at all.
     W_sc is fp16 (10-bit mantissa; scores accumulate in fp32 PSUM).
  2. LayerNorm's per-token affine commutes with everything downstream:
     the device transposes and pools the RAW hidden states and applies the
     (r_t, mu_t) correction to the 16 scores per token (score path) and as
     a rank-1 correction kappa to the pooled vector (value path):
        score[t]  = r_t * combo_raw[t] - (r_t mu_t) * corr_s[t]   (+corr_b)
        z[h]      = (sum_t es[t,h] r_t hs_raw[t] - kappa[h] 1) / D_h
     corr_s is a host-precomputed [S,H] table.  r_t = rsqrt(var+eps) is a
     3-step Newton iteration on GPSIMD (var is within a few percent of 1),
     keeping the ACT engine exp-only (no activation-table reloads) and the
     serial chain off the DVE queue.
  3. With a single query the V-projection commutes with attention pooling:
     pool first (z = es^T @ hs_raw, an fp32r [H,E] matmul against the raw
     tile), then apply W_v per head (bf16), then the out-projection (bf16).
     kappa and the softmax denominator come from one extra matmul
     es2^T @ [mu | sd] accumulated in a shared PSUM bank whose has_written
     bits are never cleared after t=0 (plain regions are per-tile columns).

Scheduling
----------
Per 128-token tile, software-pipelined: PE runs [transpose-group(t+1) |
score-chunk(t)] x4 then the pooling matmuls of t-1 (so it never waits on
the DVE combine); ACT drains transpose PSUM and runs exp; DVE runs
bn_stats and the cos/sin combine; GPSIMD runs the rsqrt chain.  All DMA is
issued on one explicitly-ordered SP stream (hidden-state tiles ahead of
the paced vw/ow prefetch; pool ring slots provide flow control).  Tiles
t >= 2 are downcast to fp16 on ACT so their transposes run at 1 cycle/row.
The output is copied and DMA'd per 256-column chunk as the out-projection
completes.

The module compiles the program once (shapes are static) and caches it.
"""

import numpy as np
import ml_dtypes

import concourse.bass as bass
import concourse.mybir as mybir
import concourse.tile as tile
from concourse import bacc, bass_utils
from concourse.masks import make_identity

F32 = mybir.dt.float32
F32R = mybir.dt.float32r
F16 = mybir.dt.float16
BF16 = mybir.dt.bfloat16

P = 128
B = 8
S = 2048
E = 2048
H = 16
D = 128
ROT = 64
R2 = ROT // 2        # 32 rotary pairs per head
PAD = 50257
EPS = 1e-5

EC = E // P          # 16 feature chunks
TC = S // P          # 16 token tiles
NA = H * R2          # 512 cols in the A (cos) block
NB = H * R2          # 512 cols in the B (sin) block
NW = 256             # weight free-dim slice for the tail projections
NO = E // NW         # 8 output-dim slices
HPW = NW // D        # heads per weight slice (2)

_CACHE = {}


def _build_program(flags):
    """Per-core SPMD program. flags: (has_corrb, has_vbias, has_obias)"""
    has_corrb, has_vbias, has_obias = flags
    nc = bacc.Bacc("TRN2", debug=False, num_devices=B)

    in_hs = nc.dram_tensor("hs", [S, E], F32R, kind="ExternalInput").ap()
    in_wa = nc.dram_tensor("wsa", [E, NA], F16, kind="ExternalInput").ap()
    in_wb = nc.dram_tensor("wsb", [E, NB], F16, kind="ExternalInput").ap()
    in_wp = nc.dram_tensor("wsp", [E, H], F16, kind="ExternalInput").ap()
    in_vw = nc.dram_tensor("vwT", [E, E], BF16, kind="ExternalInput").ap()
    in_ow = nc.dram_tensor("owT", [E, E], BF16, kind="ExternalInput").ap()
    in_ct = nc.dram_tensor("ctab", [S, R2], F32, kind="ExternalInput").ap()
    in_st = nc.dram_tensor("stab", [S, R2], F32, kind="ExternalInput").ap()
    in_co = nc.dram_tensor("corr", [S, H], F32, kind="ExternalInput").ap()
    in_cb = in_vb = in_ob = None
    if has_corrb:
        in_cb = nc.dram_tensor("corrb", [S, H], F32, kind="ExternalInput").ap()
    if has_vbias:
        in_vb = nc.dram_tensor("vbiasT", [P, EC], F32, kind="ExternalInput").ap()
    if has_obias:
        in_ob = nc.dram_tensor("obias", [1, E], F32, kind="ExternalInput").ap()
    out_t = nc.dram_tensor("out", [1, E], F32, kind="ExternalOutput").ap()

    with tile.TileContext(nc) as tc:
        with (
            tc.tile_pool(name="sing", bufs=1) as sing,
            tc.tile_pool(name="wvp", bufs=12) as wvp,
        ):
            # ---------------- constants / preloads ----------------
            ident32 = sing.tile([P, P], F32)
            make_identity(nc, ident32[:])
            ident = sing.tile([P, P], F32R)
            nc.vector.tensor_copy(out=ident[:], in_=ident32[:])
            ident_bf = sing.tile([P, P], BF16)
            nc.vector.tensor_copy(out=ident_bf[:], in_=ident32[:])
            ident_h = sing.tile([P, P], F16)
            nc.vector.tensor_copy(out=ident_h[:], in_=ident32[:])
            ctab = sing.tile([P, TC, R2], F32)
            stab = sing.tile([P, TC, R2], F32)
            corr = sing.tile([P, TC, H], F32)
            corrb = None
            if has_corrb:
                corrb = sing.tile([P, TC, H], F32)
                nc.sync.dma_start(corrb[:], in_cb.rearrange("(t p) h -> p t h", p=P))
            # fused score-projection weights, fully resident
            wsa = sing.tile([P, EC, NA], F16)
            wsb = sing.tile([P, EC, NB], F16)
            wsp = sing.tile([P, EC, H], F16)
            # prefetch the value/out projection weights; DMAs are queued
            # behind the score weights and stream in during the main loop.
            vw_tiles = []
            ow_tiles = []

            def prefetch_tail_weight(k):
                """One vw/ow chunk DMA, paced from inside the main loop so the
                prefetch never starves the hidden-state stream."""
                if k < NO:
                    vwt = wvp.tile([P, EC, NW], BF16, tag="wv", name=f"vw{k}")
                    nc.sync.dma_start(
                        vwt[:],
                        in_vw.rearrange("(ec p) o -> p ec o", p=P)[
                            :, :, k * NW:(k + 1) * NW
                        ],
                    )
                    vw_tiles.append(vwt)
                elif k < 2 * NO:
                    o = k - NO
                    owt = wvp.tile([P, EC, NW], BF16, tag="wv", name=f"ow{o}")
                    nc.sync.dma_start(
                        owt[:],
                        in_ow.rearrange("(ec p) o -> p ec o", p=P)[
                            :, :, o * NW:(o + 1) * NW
                        ],
                    )
                    ow_tiles.append(owt)

            # ---------------- main loop ----------------
            with tc.tile_pool(name="zps", bufs=1, space="PSUM") as zps:
                z_ps = [
                    zps.tile([H, 512], F32, tag=f"z{i}", name=f"z{i}")
                    for i in range(4)
                ]
                # one shared bank: 16 per-tile plain regions + [kappa|D].
                # Only two matmuls ever carry start=True in this bank (plain
                # t=0 and kd t=0), so the bank-wide has_written clear cannot
                # wipe the persistent kd accumulation.
                acc = zps.tile([P, 512], F32, tag="acc", name="acc")
                kd = acc[:H, 256:258]       # [kappa | D] accumulators (persistent)

                with (
                    tc.tile_pool(name="xtp", bufs=4) as xtp,
                    tc.tile_pool(name="lnp", bufs=3) as lnp,
                    tc.tile_pool(name="hstp", bufs=2) as hstp,
                    tc.tile_pool(name="x16p", bufs=3) as x16p,
                    tc.tile_pool(name="cmb", bufs=3) as cmb,
                    tc.tile_pool(name="cmbm", bufs=2) as cmbm,
                    tc.tile_pool(name="ptp", bufs=1, space="PSUM") as ptp,
                    tc.tile_pool(name="scp", bufs=1, space="PSUM") as scp,
                ):
                    xts = [None] * TC
                    x16s = [None] * TC
                    lns = [None] * TC
                    hsts = [None] * TC
                    ess = [None] * TC

                    def load(t):
                        xt = xtp.tile([P, E], F32R, tag="xt", name=f"xt{t}")
                        nc.sync.dma_start(xt[:], in_hs[t * P:(t + 1) * P, :])
                        xts[t] = xt

                    def ln_stats(t):
                        xt = xts[t]
                        stats = lnp.tile([P, 4, 6], F32, tag="st", name=f"st{t}")
                        for g in range(4):
                            nc.vector.bn_stats(
                                out=stats[:, g, :], in_=xt[:, g * 512:(g + 1) * 512].bitcast(F32)
                            )
                        mv = lnp.tile([P, 2], F32, tag="mv", name=f"mv{t}")
                        nc.vector.bn_aggr(out=mv[:], in_=stats[:])
                        # r = rsqrt(var + EPS) via Newton on DVE (keeps the
                        # ACT engine exp-only, avoiding act-table reloads).
                        # var is within a few percent of 1, so y0 = 1 and
                        # three iterations reach float32 roundoff.
                        # The whole serial rsqrt chain runs on the
                        # otherwise-idle GPSIMD engine: a dependent chain of
                        # tiny ops would convoy the DVE's in-order queue and
                        # stall the score combine behind it.
                        var = mv[:, 1:2]
                        vpe = lnp.tile([P, 1], F32, tag="vpe", name=f"vpe{t}")
                        r = lnp.tile([P, 1], F32, tag="r", name=f"r{t}")
                        tq = lnp.tile([P, 1], F32, tag="tq", name=f"tq{t}")
                        uq = lnp.tile([P, 1], F32, tag="uq", name=f"uq{t}")
                        nc.gpsimd.tensor_scalar_add(vpe[:], var, float(EPS))
                        nc.gpsimd.tensor_scalar_mul(r[:], vpe[:], -0.5)
                        nc.gpsimd.tensor_scalar_add(r[:], r[:], 1.5)
                        for _ in range(2):
                            nc.gpsimd.tensor_tensor(
                                tq[:], r[:], r[:], mybir.AluOpType.mult
                            )
                            nc.gpsimd.tensor_tensor(
                                uq[:], vpe[:], tq[:], mybir.AluOpType.mult
                            )
                            nc.gpsimd.tensor_scalar_mul(uq[:], uq[:], -0.5)
                            nc.gpsimd.tensor_scalar_add(uq[:], uq[:], 1.5)
                            nc.gpsimd.tensor_tensor(
                                r[:], r[:], uq[:], mybir.AluOpType.mult
                            )
                        mr = lnp.tile([P, 1], F32, tag="mr", name=f"mr{t}")
                        nc.gpsimd.tensor_tensor(
                            mr[:], mv[:, 0:1], r[:], mybir.AluOpType.mult
                        )
                        # msd = [mu | sd];  sd = (var+EPS) * r  (DVE: its F32R
                        # write must come from a rounding engine op)
                        msd = lnp.tile([P, 2], F32R, tag="msd", name=f"msd{t}")
                        nc.vector.tensor_copy(out=msd[:, 0:1], in_=mv[:, 0:1])
                        nc.vector.tensor_tensor(
                            msd[:, 1:2], vpe[:], r[:], mybir.AluOpType.mult
                        )
                        lns[t] = (mv, r, mr, msd)

                    def convert16(t):
                        """Downcast tile t to fp16 (ACT) so its transposes run
                        at 1 cycle/row instead of fp32r's 1.5."""
                        x16 = x16p.tile([P, E], F16, tag="x16", name=f"x16_{t}")
                        nc.scalar.copy(out=x16[:], in_=xts[t][:].bitcast(F32))
                        x16s[t] = x16

                    def transpose_group(t, g):
                        """4 PE transposes of feature chunks 4g..4g+3 of tile t
                        into one PSUM bank, drained by one ACT copy.  Tiles 0-1
                        transpose the raw fp32r directly (at startup there is
                        no score work to hide the fp16 downcast behind)."""
                        if g == 0:
                            hsts[t] = hstp.tile([P, E], F16, tag="hsT", name=f"hsT{t}")
                        hsT = hsts[t]
                        if t < 2:
                            pt = ptp.tile([P, 4 * P], F32R, tag="pt", name=f"pt{t}_{g}")
                            src_ap, idn = xts[t], ident
                        else:
                            pt = ptp.tile([P, 4 * P], F16, tag="pt", name=f"pt{t}_{g}")
                            src_ap, idn = x16s[t], ident_h
                        for u in range(4):
                            e = 4 * g + u
                            nc.tensor.transpose(
                                pt[:, u * P:(u + 1) * P],
                                src_ap[:, e * P:(e + 1) * P],
                                idn[:],
                            )
                        nc.scalar.copy(
                            out=hsT[:, g * 4 * P:(g + 1) * 4 * P],
                            in_=pt[:].bitcast(F32) if t < 2 else pt[:],
                        )

                    def score_chunk(t, g, sc_a, sc_b):
                        hsT = hsts[t]
                        sc_p = acc[:, t * H:(t + 1) * H]
                        for e in range(4 * g, 4 * g + 4):
                            lhs = hsT[:, e * P:(e + 1) * P]
                            nc.tensor.matmul(
                                sc_a[:], lhs, wsa[:, e, :],
                                start=(e == 0), stop=(e == EC - 1),
                            )
                            nc.tensor.matmul(
                                sc_b[:], lhs, wsb[:, e, :],
                                start=(e == 0), stop=(e == EC - 1),
                            )
                            nc.tensor.matmul(
                                sc_p, lhs, wsp[:, e, :],
                                start=(t == 0 and e == 0), stop=(e == EC - 1),
                            )

                    def combine(t, sc_a, sc_b):
                        mv, r, mr, msd = lns[t]
                        # base = sc_p * r - corr * (mu*r); reads the shared
                        # PSUM bank first so the next tile's plain matmuls
                        # never wait on this tile's combine.
                        t2 = cmb.tile([P, H], F32, tag="t2", name=f"t2_{t}")
                        nc.vector.tensor_scalar_mul(t2[:], corr[:, t, :], mr[:])
                        base = cmb.tile([P, H], F32, tag="base", name=f"base{t}")
                        nc.vector.scalar_tensor_tensor(
                            out=base[:], in0=acc[:, t * H:(t + 1) * H], scalar=r[:], in1=t2[:],
                            op0=mybir.AluOpType.mult, op1=mybir.AluOpType.subtract,
                        )
                        # cos/sin combine (DVE)
                        cb = ctab[:, t, :].unsqueeze(1).to_broadcast((P, H, R2))
                        sb = stab[:, t, :].unsqueeze(1).to_broadcast((P, H, R2))
                        m1 = cmbm.tile([P, H, R2], F32, tag="m1", name=f"m1_{t}")
                        m2 = cmbm.tile([P, H, R2], F32, tag="m2", name=f"m2_{t}")
                        nc.vector.tensor_tensor(
                            m1[:], sc_a[:].rearrange("p (h i) -> p h i", h=H), cb,
                            mybir.AluOpType.mult,
                        )
                        nc.vector.tensor_tensor(
                            m2[:], sc_b[:].rearrange("p (h i) -> p h i", h=H), sb,
                            mybir.AluOpType.mult,
                        )
                        m3 = cmbm.tile([P, H, R2], F32, tag="m3", name=f"m3_{t}")
                        nc.vector.tensor_tensor(m3[:], m1[:], m2[:], mybir.AluOpType.add)
                        red = cmb.tile([P, H], F32, tag="red", name=f"red{t}")
                        nc.vector.reduce_sum(
                            out=red[:], in_=m3[:], axis=mybir.AxisListType.X
                        )
                        sct = cmb.tile([P, H], F32, tag="sct", name=f"sct{t}")
                        nc.vector.scalar_tensor_tensor(
                            out=sct[:], in0=red[:], scalar=r[:], in1=base[:],
                            op0=mybir.AluOpType.mult, op1=mybir.AluOpType.add,
                        )
                        if has_corrb:
                            nc.vector.tensor_tensor(
                                sct[:], sct[:], corrb[:, t, :], mybir.AluOpType.add
                            )
                        es = cmb.tile([P, H], F32R, tag="es", name=f"es{t}")
                        nc.scalar.activation(
                            out=es[:], in_=sct[:],
                            func=mybir.ActivationFunctionType.Exp,
                        )
                        es2 = cmb.tile([P, H], F32R, tag="es2", name=f"es2_{t}")
                        nc.vector.tensor_scalar_mul(es2[:], es[:].bitcast(F32), r[:])
                        ess[t] = (es, es2)

                    def pool_accum(t):
                        """PE pooling matmuls for tile t (emitted one
                        iteration late so the DVE combine is long done)."""
                        xt = xts[t]
                        mv, r, mr, msd = lns[t]
                        es, es2 = ess[t]
                        for i in range(4):
                            nc.tensor.matmul(
                                z_ps[i][:], es2[:],
                                xt[:, i * 512:(i + 1) * 512],
                                start=(t == 0), stop=(t == TC - 1),
                            )
                        # [kappa | D] in one matmul: es = es2 * sd, so
                        # es2^T @ [mu | sd] = [sum es2*mu | sum es]
                        nc.tensor.matmul(
                            kd, es2[:], msd[:],
                            start=(t == 0), stop=(t == TC - 1),
                        )

                    # Software-pipelined emission.  PE queue order per tile t:
                    #   [T(t+1) group g | S(t) chunk g] x4, then Z(t-1).
                    # Transposes of t+1 fill the PSUM bank while the ACT copy
                    # of the previous group drains during the 12 score
                    # matmuls; pooling of t-1 runs while the DVE combine of t
                    # is still in flight.
                    # first two tiles go at the head of the SP DMA queue,
                    # ahead of the score weights
                    # one explicitly-ordered SP DMA stream: the FIFO
                    # gives hidden-state tiles priority over the tail-weight
                    # prefetch, and pool ring slots pace everything.
                    load(0)
                    nc.sync.dma_start(
                        wsa[:], in_wa.rearrange("(ec p) n -> p ec n", p=P))
                    load(1)
                    nc.sync.dma_start(
                        wsp[:], in_wp.rearrange("(ec p) n -> p ec n", p=P))
                    nc.sync.dma_start(
                        wsb[:], in_wb.rearrange("(ec p) n -> p ec n", p=P))
                    nc.sync.dma_start(
                        ctab[:], in_ct.rearrange("(t p) i -> p t i", p=P))
                    nc.sync.dma_start(
                        stab[:], in_st.rearrange("(t p) i -> p t i", p=P))
                    nc.sync.dma_start(
                        corr[:], in_co.rearrange("(t p) h -> p t h", p=P))
                    ln_stats(0)
                    for g in range(4):
                        transpose_group(0, g)
                    for t in range(TC):
                        if t + 2 < TC:
                            load(t + 2)
                        if t + 2 < TC:
                            convert16(t + 2)
                        sc_a = scp.tile([P, NA], F32, tag="sca", name=f"sca{t}")
                        sc_b = scp.tile([P, NB], F32, tag="scb", name=f"scb{t}")
                        for g in range(4):
                            if t + 1 < TC:
                                transpose_group(t + 1, g)
                            score_chunk(t, g, sc_a, sc_b)
                        if t >= 1:
                            pool_accum(t - 1)
                        combine(t, sc_a, sc_b)
                        if t + 1 < TC:
                            ln_stats(t + 1)
                        if t >= 7:
                            prefetch_tail_weight(t - 7)
                    pool_accum(TC - 1)
                    for k in range(TC - 7, 2 * NO):
                        prefetch_tail_weight(k)

                # ---------------- z normalization ----------------
                rd = sing.tile([H, 1], F32)
                nc.vector.reciprocal(out=rd[:], in_=kd[:, 1:2])
                kb = sing.tile([H, 1], F32)
                nc.vector.scalar_tensor_tensor(
                    out=kb[:], in0=kd[:, 0:1], scalar=-1.0, in1=rd[:],
                    op0=mybir.AluOpType.mult, op1=mybir.AluOpType.mult,
                )
                # normalize (z - kappa)/D: two chunks on DVE, two on ACT so
                # the serial drain halves
                z_sb = sing.tile([H, E], BF16)
                for i in range(4):
                    if i % 2 == 0:
                        nc.vector.tensor_scalar(
                            out=z_sb[:, i * 512:(i + 1) * 512],
                            in0=z_ps[i][:], scalar1=kd[:, 0:1], scalar2=rd[:],
                            op0=mybir.AluOpType.subtract, op1=mybir.AluOpType.mult,
                        )
                    else:
                        nc.scalar.activation(
                            out=z_sb[:, i * 512:(i + 1) * 512], in_=z_ps[i][:],
                            func=mybir.ActivationFunctionType.Identity,
                            bias=kb[:], scale=rd[:],
                        )

            # ---------------- attn-out + out-projection ----------------
            with (
                tc.tile_pool(name="fin", bufs=1) as fin,
                tc.tile_pool(name="pzp", bufs=2, space="PSUM") as pzp,
                tc.tile_pool(name="fps", bufs=1, space="PSUM") as fps,
            ):
                zT = fin.tile([P, EC, H], BF16)
                for i in range(EC):
                    pz = pzp.tile([P, H], BF16, tag="pz", name=f"pz{i}")
                    nc.tensor.transpose(
                        pz[:], z_sb[:, i * P:(i + 1) * P], ident_bf[:H, :H]
                    )
                    nc.any.tensor_copy(out=zT[:, i, :], in_=pz[:])

                # attn-out: per head-block compute all 16 head columns
                # (N=16 keeps the matmul legal) then keep the block's own
                with tc.tile_pool(name="ops", bufs=2, space="PSUM") as ops:
                    oaT = fin.tile([P, EC], BF16)
                    vbT = None
                    if has_vbias:
                        vbT = fin.tile([P, EC], F32)
                        nc.sync.dma_start(vbT[:], in_vb[:])
                    for o in range(NO):
                        vwt = vw_tiles[o]
                        for hh in range(o * HPW, (o + 1) * HPW):
                            lo = (hh - o * HPW) * D
                            op = ops.tile([P, H], F32, tag="oa", name=f"oa{hh}")
                            for i in range(EC):
                                nc.tensor.matmul(
                                    op[:],
                                    vwt[:, i, lo:lo + D],
                                    zT[:, i, :],
                                    start=(i == 0), stop=(i == EC - 1),
                                )
                            if has_vbias:
                                nc.vector.tensor_tensor(
                                    oaT[:, hh:hh + 1], op[:, hh:hh + 1],
                                    vbT[:, hh:hh + 1], mybir.AluOpType.add,
                                )
                            else:
                                nc.vector.tensor_copy(
                                    out=oaT[:, hh:hh + 1], in_=op[:, hh:hh + 1]
                                )

                # out projection: final[o] = sum_e oaT[e] * owT[e, o];
                # each o-chunk is copied out and DMA'd as soon as it stops so
                # the single-partition drain overlaps the remaining matmuls.
                f_ps = fps.tile([1, E], F32, tag="fo")
                f_sb = fin.tile([1, E], F32)
                ob_t = None
                if has_obias:
                    ob_t = fin.tile([1, E], F32)
                    nc.sync.dma_start(ob_t[:], in_ob[:])
                for o in range(NO):
                    owt = ow_tiles[o]
                    for e in range(EC):
                        nc.tensor.matmul(
                            f_ps[:, o * NW:(o + 1) * NW],
                            oaT[:, e:e + 1],
                            owt[:, e, :],
                            start=(e == 0), stop=(e == EC - 1),
                        )
                    sl = slice(o * NW, (o + 1) * NW)
                    if has_obias:
                        nc.vector.tensor_tensor(
                            f_sb[:, sl], f_ps[:, sl], ob_t[:, sl],
                            mybir.AluOpType.add,
                        )
                    else:
                        nc.vector.tensor_copy(out=f_sb[:, sl], in_=f_ps[:, sl])
                    nc.sync.dma_start(out_t[:, sl], f_sb[:, sl])

    nc.compile()
    return nc


def _prep_host(inputs):
    hs = np.ascontiguousarray(np.asarray(inputs["hidden_states"], dtype=np.float32))
    ids = np.asarray(inputs["input_ids_with_pads"])
    ln_w = np.asarray(inputs["ln_w"], dtype=np.float64)
    ln_b = np.asarray(inputs["ln_b"], dtype=np.float64)
    k_w = np.asarray(inputs["k_w"], dtype=np.float64)
    q_w = np.asarray(inputs["q_w"], dtype=np.float64)
    v_w = np.asarray(inputs["v_w"], dtype=np.float32)
    o_w = np.asarray(inputs["out_w"], dtype=np.float32)
    k_b = np.asarray(inputs["k_b"], dtype=np.float64)
    q_b = np.asarray(inputs["q_b"], dtype=np.float64)
    v_b = np.asarray(inputs["v_b"], dtype=np.float64)
    o_b = np.asarray(inputs["out_b"], dtype=np.float32)

    # last non-pad token index per row
    ix = np.argmax(np.cumsum((ids != PAD).astype(np.int64), axis=1), axis=1)

    # exact q per batch row (host-side: tiny)
    hl = hs[np.arange(B), ix].astype(np.float64)            # [B,E]
    mu = hl.mean(-1, keepdims=True)
    var = ((hl - mu) ** 2).mean(-1, keepdims=True)
    hln = (hl - mu) / np.sqrt(var + EPS) * ln_w + ln_b
    q = hln @ q_w.T + q_b                                    # [B,E]

    # rotary tables
    inv = 1.0 / (10000.0 ** (np.arange(0, ROT, 2, dtype=np.float64) / ROT))
    ang = np.arange(S, dtype=np.float64)[:, None] * inv[None, :]
    cosd, sind = np.cos(ang), np.sin(ang)                    # [S,R2]

    # fused score-projection per batch: W_sc = [A | B | plain]
    Wk3 = (k_w * ln_w[None, :]).reshape(H, D, E)             # [H,D,E]
    Wp = Wk3[:, :ROT].reshape(H, R2, 2, E)
    q3 = q.reshape(B, H, D)
    qp = q3[:, :, :ROT].reshape(B, H, R2, 2)
    A = qp[..., 0, None] * Wp[None, :, :, 0] + qp[..., 1, None] * Wp[None, :, :, 1]
    Bm = qp[..., 1, None] * Wp[None, :, :, 0] - qp[..., 0, None] * Wp[None, :, :, 1]
    Pl = np.einsum('bhd,hde->bhe', q3[:, :, ROT:], Wk3[:, ROT:])   # [B,H,E]
    sA, sB, sP = A.sum(-1), Bm.sum(-1), Pl.sum(-1)
    corr = (np.einsum('si,bhi->bsh', cosd, sA)
            + np.einsum('si,bhi->bsh', sind, sB) + sP[:, None, :])  # [B,S,H]

    # k-bias contribution (constant per (t,h) score offset)
    kb_eff = (ln_b @ k_w.T + k_b).reshape(H, D)
    kbp = kb_eff[:, :ROT].reshape(H, R2, 2)
    biasA = qp[..., 0] * kbp[None, ..., 0] + qp[..., 1] * kbp[None, ..., 1]
    biasB = qp[..., 1] * kbp[None, ..., 0] - qp[..., 0] * kbp[None, ..., 1]
    biasP = np.einsum('bhd,hd->bh', q3[:, :, ROT:], kb_eff[:, ROT:])
    corrb = (np.einsum('si,bhi->bsh', cosd, biasA)
             + np.einsum('si,bhi->bsh', sind, biasB) + biasP[:, None, :])

    vwT = np.ascontiguousarray(
        (v_w * np.float32(1) * ln_w[None, :].astype(np.float32)).T
        .astype(ml_dtypes.bfloat16))
    owT = np.ascontiguousarray(o_w.T.astype(ml_dtypes.bfloat16))
    vbias = (ln_b @ v_w.T.astype(np.float64) + v_b)
    obias = o_b

    flags = (
        bool(np.any(corrb)), bool(np.any(vbias)), bool(np.any(obias)),
    )

    shared = {
        "vwT": vwT, "owT": owT,
        "ctab": np.ascontiguousarray(cosd.astype(np.float32)),
        "stab": np.ascontiguousarray(sind.astype(np.float32)),
    }
    if flags[1]:
        shared["vbiasT"] = np.ascontiguousarray(
            vbias.astype(np.float32).reshape(EC, P).T)
    if flags[2]:
        shared["obias"] = np.ascontiguousarray(obias[None, :])

    in_maps = []
    for b in range(B):
        m = dict(shared)
        m["hs"] = np.ascontiguousarray(hs[b])
        m["wsa"] = np.ascontiguousarray(
            A[b].transpose(2, 0, 1).reshape(E, NA).astype(np.float16))
        m["wsb"] = np.ascontiguousarray(
            Bm[b].transpose(2, 0, 1).reshape(E, NB).astype(np.float16))
        m["wsp"] = np.ascontiguousarray(Pl[b].T.astype(np.float16))
        m["corr"] = np.ascontiguousarray(corr[b].astype(np.float32))
        if flags[0]:
            m["corrb"] = np.ascontiguousarray(corrb[b].astype(np.float32))
        in_maps.append(m)
    return flags, in_maps


def kernel(**inputs):
    flags, in_maps = _prep_host(inputs)
    if flags not in _CACHE:
        _CACHE[flags] = _build_program(flags)
    nc = _CACHE[flags]
    res = bass_utils.run_bass_kernel_spmd(nc, in_maps, core_ids=list(range(B)))
    out = np.stack([res.results[b]["out"][0] for b in range(B)], axis=0)
    return out.astype(np.float32)


# revision 50
# speedup vs baseline: 1.0369x; 1.0018x over previous
"""NostARHead attention kernel for Trainium2 (8 NeuronCores, batch-parallel).

Strategy
--------
Data-parallel over batch: core b handles batch element b (B == n_cores == 8).

Algebraic structure exploited:
  1. The single query token sits at rotary position 0, where the RoPE
     rotation is the identity (sin 0 = 0, cos 0 = 1).  The attention score
     therefore factors through a fixed basis of cos/sin position features:
        score[t,h] = sum_j cos(th_j t) * a[t,h,j] + sin(th_j t) * b[t,h,j]
                     + plain[t,h]
     where a, b, plain are LINEAR in hs[t] with weights combining q and W_k.
     Since the last-token index (and hence q) is host-computable, the host
     builds a fused score-projection matrix W_sc = [A | B | plain] of shape
     [E, 2*H*R2 + H] = [2048, 1040] -- roughly HALF the FLOPs of the full
     K-projection, with no on-device RoPE and no on-device q-path at all.
     W_sc is fp16 (10-bit mantissa; scores accumulate in fp32 PSUM).
  2. LayerNorm's per-token affine commutes with everything downstream:
     the device transposes and pools the RAW hidden states and applies the
     (r_t, mu_t) correction to the 16 scores per token (score path) and as
     a rank-1 correction kappa to the pooled vector (value path):
        score[t]  = r_t * combo_raw[t] - (r_t mu_t) * corr_s[t]   (+corr_b)
        z[h]      = (sum_t es[t,h] r_t hs_raw[t] - kappa[h] 1) / D_h
     corr_s is a host-precomputed [S,H] table.  r_t = rsqrt(var+eps) is a
     3-step Newton iteration on GPSIMD (var is within a few percent of 1),
     keeping the ACT engine exp-only (no activation-table reloads) and the
     serial chain off the DVE queue.
  3. With a single query the V-projection commutes with attention pooling:
     pool first (z = es^T @ hs_raw, an fp32r [H,E] matmul against the raw
     tile), then apply W_v per head (bf16), then the out-projection (bf16).
     kappa and the softmax denominator come from one extra matmul
     es2^T @ [mu | sd] accumulated in a shared PSUM bank whose has_written
     bits are never cleared after t=0 (plain regions are per-tile columns).

Scheduling
----------
Per 128-token tile, software-pipelined: PE runs [transpose-group(t+1) |
score-chunk(t)] x4 then the pooling matmuls of t-1 (so it never waits on
the DVE combine); ACT drains transpose PSUM and runs exp; DVE runs
bn_stats and the cos/sin combine; GPSIMD runs the rsqrt chain.  All DMA is
issued on one explicitly-ordered SP stream (hidden-state tiles ahead of
the paced vw/ow prefetch; pool ring slots provide flow control).  Tiles
t >= 2 are downcast to fp16 on ACT so their transposes run at 1 cycle/row.
The output is copied and DMA'd per 256-column chunk as the out-projection
completes.

The module compiles the program once (shapes are static) and caches it.
"""

import numpy as np
import ml_dtypes

import concourse.bass as bass
import concourse.mybir as mybir
import concourse.tile as tile
from concourse import bacc, bass_utils
from concourse.masks import make_identity

F32 = mybir.dt.float32
F32R = mybir.dt.float32r
F16 = mybir.dt.float16
BF16 = mybir.dt.bfloat16

P = 128
B = 8
S = 2048
E = 2048
H = 16
D = 128
ROT = 64
R2 = ROT // 2        # 32 rotary pairs per head
PAD = 50257
EPS = 1e-5

EC = E // P          # 16 feature chunks
TC = S // P          # 16 token tiles
NA = H * R2          # 512 cols in the A (cos) block
NB = H * R2          # 512 cols in the B (sin) block
NW = 256             # weight free-dim slice for the tail projections
NO = E // NW         # 8 output-dim slices
HPW = NW // D        # heads per weight slice (2)

_CACHE = {}


def _build_program(flags):
    """Per-core SPMD program. flags: (has_corrb, has_vbias, has_obias)"""
    has_corrb, has_vbias, has_obias = flags
    nc = bacc.Bacc("TRN2", debug=False, num_devices=B)

    in_hs = nc.dram_tensor("hs", [S, E], F32R, kind="ExternalInput").ap()
    in_wa = nc.dram_tensor("wsa", [E, NA], F16, kind="ExternalInput").ap()
    in_wb = nc.dram_tensor("wsb", [E, NB], F16, kind="ExternalInput").ap()
    in_wp = nc.dram_tensor("wsp", [E, H], F16, kind="ExternalInput").ap()
    in_vw = nc.dram_tensor("vwT", [E, E], BF16, kind="ExternalInput").ap()
    in_ow = nc.dram_tensor("owT", [E, E], BF16, kind="ExternalInput").ap()
    in_ct = nc.dram_tensor("ctab", [S, R2], F32, kind="ExternalInput").ap()
    in_st = nc.dram_tensor("stab", [S, R2], F32, kind="ExternalInput").ap()
    in_co = nc.dram_tensor("corr", [S, H], F32, kind="ExternalInput").ap()
    in_cb = in_vb = in_ob = None
    if has_corrb:
        in_cb = nc.dram_tensor("corrb", [S, H], F32, kind="ExternalInput").ap()
    if has_vbias:
        in_vb = nc.dram_tensor("vbiasT", [P, EC], F32, kind="ExternalInput").ap()
    if has_obias:
        in_ob = nc.dram_tensor("obias", [1, E], F32, kind="ExternalInput").ap()
    out_t = nc.dram_tensor("out", [1, E], F32, kind="ExternalOutput").ap()

    with tile.TileContext(nc) as tc:
        with (
            tc.tile_pool(name="sing", bufs=1) as sing,
            tc.tile_pool(name="wvp", bufs=12) as wvp,
        ):
            # ---------------- constants / preloads ----------------
            ident32 = sing.tile([P, P], F32)
            make_identity(nc, ident32[:])
            ident = sing.tile([P, P], F32R)
            nc.vector.tensor_copy(out=ident[:], in_=ident32[:])
            ident_bf = sing.tile([P, P], BF16)
            nc.vector.tensor_copy(out=ident_bf[:], in_=ident32[:])
            ident_h = sing.tile([P, P], F16)
            nc.vector.tensor_copy(out=ident_h[:], in_=ident32[:])
            ctab = sing.tile([P, TC, R2], F32)
            stab = sing.tile([P, TC, R2], F32)
            corr = sing.tile([P, TC, H], F32)
            corrb = None
            if has_corrb:
                corrb = sing.tile([P, TC, H], F32)
                nc.sync.dma_start(corrb[:], in_cb.rearrange("(t p) h -> p t h", p=P))
            # fused score-projection weights, fully resident
            wsa = sing.tile([P, EC, NA], F16)
            wsb = sing.tile([P, EC, NB], F16)
            wsp = sing.tile([P, EC, H], F16)
            # prefetch the value/out projection weights; DMAs are queued
            # behind the score weights and stream in during the main loop.
            vw_tiles = []
            ow_tiles = []

            def prefetch_tail_weight(k):
                """One vw/ow chunk DMA, paced from inside the main loop so the
                prefetch never starves the hidden-state stream."""
                if k < NO:
                    vwt = wvp.tile([P, EC, NW], BF16, tag="wv", name=f"vw{k}")
                    nc.sync.dma_start(
                        vwt[:],
                        in_vw.rearrange("(ec p) o -> p ec o", p=P)[
                            :, :, k * NW:(k + 1) * NW
                        ],
                    )
                    vw_tiles.append(vwt)
                elif k < 2 * NO:
                    o = k - NO
                    owt = wvp.tile([P, EC, NW], BF16, tag="wv", name=f"ow{o}")
                    nc.sync.dma_start(
                        owt[:],
                        in_ow.rearrange("(ec p) o -> p ec o", p=P)[
                            :, :, o * NW:(o + 1) * NW
                        ],
                    )
                    ow_tiles.append(owt)

            # ---------------- main loop ----------------
            with tc.tile_pool(name="zps", bufs=1, space="PSUM") as zps:
                z_ps = [
                    zps.tile([H, 512], F32, tag=f"z{i}", name=f"z{i}")
                    for i in range(4)
                ]
                # one shared bank: 16 per-tile plain regions + [kappa|D].
                # Only two matmuls ever carry start=True in this bank (plain
                # t=0 and kd t=0), so the bank-wide has_written clear cannot
                # wipe the persistent kd accumulation.
                acc = zps.tile([P, 512], F32, tag="acc", name="acc")
                kd = acc[:H, 256:258]       # [kappa | D] accumulators (persistent)

                with (
                    tc.tile_pool(name="xtp", bufs=4) as xtp,
                    tc.tile_pool(name="lnp", bufs=3) as lnp,
                    tc.tile_pool(name="hstp", bufs=2) as hstp,
                    tc.tile_pool(name="x16p", bufs=3) as x16p,
                    tc.tile_pool(name="cmb", bufs=3) as cmb,
                    tc.tile_pool(name="cmbm", bufs=2) as cmbm,
                    tc.tile_pool(name="ptp", bufs=1, space="PSUM") as ptp,
                    tc.tile_pool(name="scp", bufs=1, space="PSUM") as scp,
                ):
                    xts = [None] * TC
                    x16s = [None] * TC
                    lns = [None] * TC
                    hsts = [None] * TC
                    ess = [None] * TC

                    def load(t):
                        xt = xtp.tile([P, E], F32R, tag="xt", name=f"xt{t}")
                        nc.sync.dma_start(xt[:], in_hs[t * P:(t + 1) * P, :])
                        xts[t] = xt

                    def ln_stats(t):
                        xt = xts[t]
                        stats = lnp.tile([P, 4, 6], F32, tag="st", name=f"st{t}")
                        for g in range(4):
                            nc.vector.bn_stats(
                                out=stats[:, g, :], in_=xt[:, g * 512:(g + 1) * 512].bitcast(F32)
                            )
                        mv = lnp.tile([P, 2], F32, tag="mv", name=f"mv{t}")
                        nc.vector.bn_aggr(out=mv[:], in_=stats[:])
                        # r = rsqrt(var + EPS) via Newton on DVE (keeps the
                        # ACT engine exp-only, avoiding act-table reloads).
                        # var is within a few percent of 1, so y0 = 1 and
                        # three iterations reach float32 roundoff.
                        # The whole serial rsqrt chain runs on the
                        # otherwise-idle GPSIMD engine: a dependent chain of
                        # tiny ops would convoy the DVE's in-order queue and
                        # stall the score combine behind it.
                        var = mv[:, 1:2]
                        vpe = lnp.tile([P, 1], F32, tag="vpe", name=f"vpe{t}")
                        r = lnp.tile([P, 1], F32, tag="r", name=f"r{t}")
                        tq = lnp.tile([P, 1], F32, tag="tq", name=f"tq{t}")
                        uq = lnp.tile([P, 1], F32, tag="uq", name=f"uq{t}")
                        nc.gpsimd.tensor_scalar_add(vpe[:], var, float(EPS))
                        nc.gpsimd.tensor_scalar_mul(r[:], vpe[:], -0.5)
                        nc.gpsimd.tensor_scalar_add(r[:], r[:], 1.5)
                        for _ in range(2):
                            nc.gpsimd.tensor_tensor(
                                tq[:], r[:], r[:], mybir.AluOpType.mult
                            )
                            nc.gpsimd.tensor_tensor(
                                uq[:], vpe[:], tq[:], mybir.AluOpType.mult
                            )
                            nc.gpsimd.tensor_scalar_mul(uq[:], uq[:], -0.5)
                            nc.gpsimd.tensor_scalar_add(uq[:], uq[:], 1.5)
                            nc.gpsimd.tensor_tensor(
                                r[:], r[:], uq[:], mybir.AluOpType.mult
                            )
                        mr = lnp.tile([P, 1], F32, tag="mr", name=f"mr{t}")
                        nc.gpsimd.tensor_tensor(
                            mr[:], mv[:, 0:1], r[:], mybir.AluOpType.mult
                        )
                        # msd = [mu | sd];  sd = (var+EPS) * r  (DVE: its F32R
                        # write must come from a rounding engine op)
                        msd = lnp.tile([P, 2], F32R, tag="msd", name=f"msd{t}")
                        nc.vector.tensor_copy(out=msd[:, 0:1], in_=mv[:, 0:1])
                        nc.vector.tensor_tensor(
                            msd[:, 1:2], vpe[:], r[:], mybir.AluOpType.mult
                        )
                        lns[t] = (mv, r, mr, msd)

                    def convert16(t):
                        """Downcast tile t to fp16 (ACT) so its transposes run
                        at 1 cycle/row instead of fp32r's 1.5."""
                        x16 = x16p.tile([P, E], F16, tag="x16", name=f"x16_{t}")
                        nc.scalar.copy(out=x16[:], in_=xts[t][:].bitcast(F32))
                        x16s[t] = x16

                    def transpose_group(t, g):
                        """4 PE transposes of feature chunks 4g..4g+3 of tile t
                        into one PSUM bank, drained by one ACT copy.  Tiles 0-1
                        transpose the raw fp32r directly (at startup there is
                        no score work to hide the fp16 downcast behind)."""
                        if g == 0:
                            hsts[t] = hstp.tile([P, E], F16, tag="hsT", name=f"hsT{t}")
                        hsT = hsts[t]
                        if t < 2:
                            pt = ptp.tile([P, 4 * P], F32R, tag="pt", name=f"pt{t}_{g}")
                            src_ap, idn = xts[t], ident
                        else:
                            pt = ptp.tile([P, 4 * P], F16, tag="pt", name=f"pt{t}_{g}")
                            src_ap, idn = x16s[t], ident_h
                        for u in range(4):
                            e = 4 * g + u
                            nc.tensor.transpose(
                                pt[:, u * P:(u + 1) * P],
                                src_ap[:, e * P:(e + 1) * P],
                                idn[:],
                            )
                        nc.scalar.copy(
                            out=hsT[:, g * 4 * P:(g + 1) * 4 * P],
                            in_=pt[:].bitcast(F32) if t < 2 else pt[:],
                        )

                    def score_chunk(t, g, sc_a, sc_b):
                        # sub-chunks 0..3: A-block + plain; 4..7: B-block
                        hsT = hsts[t]
                        sc_p = acc[:, t * H:(t + 1) * H]
                        if g < 4:
                            for e in range(4 * g, 4 * g + 4):
                                lhs = hsT[:, e * P:(e + 1) * P]
                                nc.tensor.matmul(
                                    sc_a[:], lhs, wsa[:, e, :],
                                    start=(e == 0), stop=(e == EC - 1),
                                )
                                nc.tensor.matmul(
                                    sc_p, lhs, wsp[:, e, :],
                                    start=(t == 0 and e == 0), stop=(e == EC - 1),
                                )
                        else:
                            for e in range(4 * (g - 4), 4 * (g - 4) + 4):
                                lhs = hsT[:, e * P:(e + 1) * P]
                                nc.tensor.matmul(
                                    sc_b[:], lhs, wsb[:, e, :],
                                    start=(e == 0), stop=(e == EC - 1),
                                )

                    def combine(t, sc_a, sc_b):
                        mv, r, mr, msd = lns[t]
                        # base = sc_p * r - corr * (mu*r); reads the shared
                        # PSUM bank first so the next tile's plain matmuls
                        # never wait on this tile's combine.
                        t2 = cmb.tile([P, H], F32, tag="t2", name=f"t2_{t}")
                        nc.vector.tensor_scalar_mul(t2[:], corr[:, t, :], mr[:])
                        base = cmb.tile([P, H], F32, tag="base", name=f"base{t}")
                        nc.vector.scalar_tensor_tensor(
                            out=base[:], in0=acc[:, t * H:(t + 1) * H], scalar=r[:], in1=t2[:],
                            op0=mybir.AluOpType.mult, op1=mybir.AluOpType.subtract,
                        )
                        # cos/sin combine (DVE)
                        cb = ctab[:, t, :].unsqueeze(1).to_broadcast((P, H, R2))
                        sb = stab[:, t, :].unsqueeze(1).to_broadcast((P, H, R2))
                        m1 = cmbm.tile([P, H, R2], F32, tag="m1", name=f"m1_{t}")
                        m2 = cmbm.tile([P, H, R2], F32, tag="m2", name=f"m2_{t}")
                        nc.vector.tensor_tensor(
                            m1[:], sc_a[:].rearrange("p (h i) -> p h i", h=H), cb,
                            mybir.AluOpType.mult,
                        )
                        nc.vector.tensor_tensor(
                            m2[:], sc_b[:].rearrange("p (h i) -> p h i", h=H), sb,
                            mybir.AluOpType.mult,
                        )
                        m3 = cmbm.tile([P, H, R2], F32, tag="m3", name=f"m3_{t}")
                        nc.vector.tensor_tensor(m3[:], m1[:], m2[:], mybir.AluOpType.add)
                        red = cmb.tile([P, H], F32, tag="red", name=f"red{t}")
                        nc.vector.reduce_sum(
                            out=red[:], in_=m3[:], axis=mybir.AxisListType.X
                        )
                        sct = cmb.tile([P, H], F32, tag="sct", name=f"sct{t}")
                        nc.vector.scalar_tensor_tensor(
                            out=sct[:], in0=red[:], scalar=r[:], in1=base[:],
                            op0=mybir.AluOpType.mult, op1=mybir.AluOpType.add,
                        )
                        if has_corrb:
                            nc.vector.tensor_tensor(
                                sct[:], sct[:], corrb[:, t, :], mybir.AluOpType.add
                            )
                        es = cmb.tile([P, H], F32R, tag="es", name=f"es{t}")
                        nc.scalar.activation(
                            out=es[:], in_=sct[:],
                            func=mybir.ActivationFunctionType.Exp,
                        )
                        es2 = cmb.tile([P, H], F32R, tag="es2", name=f"es2_{t}")
                        nc.vector.tensor_scalar_mul(es2[:], es[:].bitcast(F32), r[:])
                        ess[t] = (es, es2)

                    def pool_accum(t):
                        """PE pooling matmuls for tile t (emitted one
                        iteration late so the DVE combine is long done)."""
                        xt = xts[t]
                        mv, r, mr, msd = lns[t]
                        es, es2 = ess[t]
                        for i in range(4):
                            nc.tensor.matmul(
                                z_ps[i][:], es2[:],
                                xt[:, i * 512:(i + 1) * 512],
                                start=(t == 0), stop=(t == TC - 1),
                            )
                        # [kappa | D] in one matmul: es = es2 * sd, so
                        # es2^T @ [mu | sd] = [sum es2*mu | sum es]
                        nc.tensor.matmul(
                            kd, es2[:], msd[:],
                            start=(t == 0), stop=(t == TC - 1),
                        )

                    # Software-pipelined emission.  PE queue order per tile t:
                    #   [T(t+1) group g | S(t) chunk g] x4, then Z(t-1).
                    # Transposes of t+1 fill the PSUM bank while the ACT copy
                    # of the previous group drains during the 12 score
                    # matmuls; pooling of t-1 runs while the DVE combine of t
                    # is still in flight.
                    # first two tiles go at the head of the SP DMA queue,
                    # ahead of the score weights
                    # one explicitly-ordered SP DMA stream: the FIFO
                    # gives hidden-state tiles priority over the tail-weight
                    # prefetch, and pool ring slots pace everything.
                    load(0)
                    nc.sync.dma_start(
                        wsa[:], in_wa.rearrange("(ec p) n -> p ec n", p=P))
                    load(1)
                    nc.sync.dma_start(
                        wsp[:], in_wp.rearrange("(ec p) n -> p ec n", p=P))
                    nc.sync.dma_start(
                        wsb[:], in_wb.rearrange("(ec p) n -> p ec n", p=P))
                    nc.sync.dma_start(
                        ctab[:], in_ct.rearrange("(t p) i -> p t i", p=P))
                    nc.sync.dma_start(
                        stab[:], in_st.rearrange("(t p) i -> p t i", p=P))
                    nc.sync.dma_start(
                        corr[:], in_co.rearrange("(t p) h -> p t h", p=P))
                    ln_stats(0)
                    for g in range(4):
                        transpose_group(0, g)
                    for t in range(TC):
                        if t + 2 < TC:
                            load(t + 2)
                        if t + 2 < TC:
                            convert16(t + 2)
                        sc_a = scp.tile([P, NA], F32, tag="sca", name=f"sca{t}")
                        sc_b = scp.tile([P, NB], F32, tag="scb", name=f"scb{t}")
                        for g in range(8):
                            if t + 1 < TC and g % 2 == 0:
                                transpose_group(t + 1, g // 2)
                            score_chunk(t, g, sc_a, sc_b)
                        if t >= 1:
                            pool_accum(t - 1)
                        combine(t, sc_a, sc_b)
                        if t + 1 < TC:
                            ln_stats(t + 1)
                        if t >= 7:
                            prefetch_tail_weight(t - 7)
                    pool_accum(TC - 1)
                    for k in range(TC - 7, 2 * NO):
                        prefetch_tail_weight(k)

                # ---------------- z normalization ----------------
                rd = sing.tile([H, 1], F32)
                nc.vector.reciprocal(out=rd[:], in_=kd[:, 1:2])
                kb = sing.tile([H, 1], F32)
                nc.vector.scalar_tensor_tensor(
                    out=kb[:], in0=kd[:, 0:1], scalar=-1.0, in1=rd[:],
                    op0=mybir.AluOpType.mult, op1=mybir.AluOpType.mult,
                )
                # normalize (z - kappa)/D: two chunks on DVE, two on ACT so
                # the serial drain halves
                z_sb = sing.tile([H, E], BF16)
                for i in range(4):
                    if i % 2 == 0:
                        nc.vector.tensor_scalar(
                            out=z_sb[:, i * 512:(i + 1) * 512],
                            in0=z_ps[i][:], scalar1=kd[:, 0:1], scalar2=rd[:],
                            op0=mybir.AluOpType.subtract, op1=mybir.AluOpType.mult,
                        )
                    else:
                        nc.scalar.activation(
                            out=z_sb[:, i * 512:(i + 1) * 512], in_=z_ps[i][:],
                            func=mybir.ActivationFunctionType.Identity,
                            bias=kb[:], scale=rd[:],
                        )

            # ---------------- attn-out + out-projection ----------------
            with (
                tc.tile_pool(name="fin", bufs=1) as fin,
                tc.tile_pool(name="pzp", bufs=2, space="PSUM") as pzp,
                tc.tile_pool(name="fps", bufs=1, space="PSUM") as fps,
            ):
                zT = fin.tile([P, EC, H], BF16)
                for i in range(EC):
                    pz = pzp.tile([P, H], BF16, tag="pz", name=f"pz{i}")
                    nc.tensor.transpose(
                        pz[:], z_sb[:, i * P:(i + 1) * P], ident_bf[:H, :H]
                    )
                    nc.any.tensor_copy(out=zT[:, i, :], in_=pz[:])

                # attn-out: per head-block compute all 16 head columns
                # (N=16 keeps the matmul legal) then keep the block's own
                with tc.tile_pool(name="ops", bufs=2, space="PSUM") as ops:
                    oaT = fin.tile([P, EC], BF16)
                    vbT = None
                    if has_vbias:
                        vbT = fin.tile([P, EC], F32)
                        nc.sync.dma_start(vbT[:], in_vb[:])
                    for o in range(NO):
                        vwt = vw_tiles[o]
                        for hh in range(o * HPW, (o + 1) * HPW):
                            lo = (hh - o * HPW) * D
                            op = ops.tile([P, H], F32, tag="oa", name=f"oa{hh}")
                            for i in range(EC):
                                nc.tensor.matmul(
                                    op[:],
                                    vwt[:, i, lo:lo + D],
                                    zT[:, i, :],
                                    start=(i == 0), stop=(i == EC - 1),
                                )
                            if has_vbias:
                                nc.vector.tensor_tensor(
                                    oaT[:, hh:hh + 1], op[:, hh:hh + 1],
                                    vbT[:, hh:hh + 1], mybir.AluOpType.add,
                                )
                            else:
                                nc.vector.tensor_copy(
                                    out=oaT[:, hh:hh + 1], in_=op[:, hh:hh + 1]
                                )

                # out projection: final[o] = sum_e oaT[e] * owT[e, o];
                # each o-chunk is copied out and DMA'd as soon as it stops so
                # the single-partition drain overlaps the remaining matmuls.
                f_ps = fps.tile([1, E], F32, tag="fo")
                f_sb = fin.tile([1, E], F32)
                ob_t = None
                if has_obias:
                    ob_t = fin.tile([1, E], F32)
                    nc.sync.dma_start(ob_t[:], in_ob[:])
                for o in range(NO):
                    owt = ow_tiles[o]
                    for e in range(EC):
                        nc.tensor.matmul(
                            f_ps[:, o * NW:(o + 1) * NW],
                            oaT[:, e:e + 1],
                            owt[:, e, :],
                            start=(e == 0), stop=(e == EC - 1),
                        )
                    sl = slice(o * NW, (o + 1) * NW)
                    if has_obias:
                        nc.vector.tensor_tensor(
                            f_sb[:, sl], f_ps[:, sl], ob_t[:, sl],
                            mybir.AluOpType.add,
                        )
                    else:
                        nc.vector.tensor_copy(out=f_sb[:, sl], in_=f_ps[:, sl])
                    nc.sync.dma_start(out_t[:, sl], f_sb[:, sl])

    nc.compile()
    return nc


def _prep_host(inputs):
    hs = np.ascontiguousarray(np.asarray(inputs["hidden_states"], dtype=np.float32))
    ids = np.asarray(inputs["input_ids_with_pads"])
    ln_w = np.asarray(inputs["ln_w"], dtype=np.float64)
    ln_b = np.asarray(inputs["ln_b"], dtype=np.float64)
    k_w = np.asarray(inputs["k_w"], dtype=np.float64)
    q_w = np.asarray(inputs["q_w"], dtype=np.float64)
    v_w = np.asarray(inputs["v_w"], dtype=np.float32)
    o_w = np.asarray(inputs["out_w"], dtype=np.float32)
    k_b = np.asarray(inputs["k_b"], dtype=np.float64)
    q_b = np.asarray(inputs["q_b"], dtype=np.float64)
    v_b = np.asarray(inputs["v_b"], dtype=np.float64)
    o_b = np.asarray(inputs["out_b"], dtype=np.float32)

    # last non-pad token index per row
    ix = np.argmax(np.cumsum((ids != PAD).astype(np.int64), axis=1), axis=1)

    # exact q per batch row (host-side: tiny)
    hl = hs[np.arange(B), ix].astype(np.float64)            # [B,E]
    mu = hl.mean(-1, keepdims=True)
    var = ((hl - mu) ** 2).mean(-1, keepdims=True)
    hln = (hl - mu) / np.sqrt(var + EPS) * ln_w + ln_b
    q = hln @ q_w.T + q_b                                    # [B,E]

    # rotary tables
    inv = 1.0 / (10000.0 ** (np.arange(0, ROT, 2, dtype=np.float64) / ROT))
    ang = np.arange(S, dtype=np.float64)[:, None] * inv[None, :]
    cosd, sind = np.cos(ang), np.sin(ang)                    # [S,R2]

    # fused score-projection per batch: W_sc = [A | B | plain]
    Wk3 = (k_w * ln_w[None, :]).reshape(H, D, E)             # [H,D,E]
    Wp = Wk3[:, :ROT].reshape(H, R2, 2, E)
    q3 = q.reshape(B, H, D)
    qp = q3[:, :, :ROT].reshape(B, H, R2, 2)
    A = qp[..., 0, None] * Wp[None, :, :, 0] + qp[..., 1, None] * Wp[None, :, :, 1]
    Bm = qp[..., 1, None] * Wp[None, :, :, 0] - qp[..., 0, None] * Wp[None, :, :, 1]
    Pl = np.einsum('bhd,hde->bhe', q3[:, :, ROT:], Wk3[:, ROT:])   # [B,H,E]
    sA, sB, sP = A.sum(-1), Bm.sum(-1), Pl.sum(-1)
    corr = (np.einsum('si,bhi->bsh', cosd, sA)
            + np.einsum('si,bhi->bsh', sind, sB) + sP[:, None, :])  # [B,S,H]

    # k-bias contribution (constant per (t,h) score offset)
    kb_eff = (ln_b @ k_w.T + k_b).reshape(H, D)
    kbp = kb_eff[:, :ROT].reshape(H, R2, 2)
    biasA = qp[..., 0] * kbp[None, ..., 0] + qp[..., 1] * kbp[None, ..., 1]
    biasB = qp[..., 1] * kbp[None, ..., 0] - qp[..., 0] * kbp[None, ..., 1]
    biasP = np.einsum('bhd,hd->bh', q3[:, :, ROT:], kb_eff[:, ROT:])
    corrb = (np.einsum('si,bhi->bsh', cosd, biasA)
             + np.einsum('si,bhi->bsh', sind, biasB) + biasP[:, None, :])

    vwT = np.ascontiguousarray(
        (v_w * np.float32(1) * ln_w[None, :].astype(np.float32)).T
        .astype(ml_dtypes.bfloat16))
    owT = np.ascontiguousarray(o_w.T.astype(ml_dtypes.bfloat16))
    vbias = (ln_b @ v_w.T.astype(np.float64) + v_b)
    obias = o_b

    flags = (
        bool(np.any(corrb)), bool(np.any(vbias)), bool(np.any(obias)),
    )

    shared = {
        "vwT": vwT, "owT": owT,
        "ctab": np.ascontiguousarray(cosd.astype(np.float32)),
        "stab": np.ascontiguousarray(sind.astype(np.float32)),
    }
    if flags[1]:
        shared["vbiasT"] = np.ascontiguousarray(
            vbias.astype(np.float32).reshape(EC, P).T)
    if flags[2]:
        shared["obias"] = np.ascontiguousarray(obias[None, :])

    in_maps = []
    for b in range(B):
        m = dict(shared)
        m["hs"] = np.ascontiguousarray(hs[b])
        m["wsa"] = np.ascontiguousarray(
            A[b].transpose(2, 0, 1).reshape(E, NA).astype(np.float16))
        m["wsb"] = np.ascontiguousarray(
            Bm[b].transpose(2, 0, 1).reshape(E, NB).astype(np.float16))
        m["wsp"] = np.ascontiguousarray(Pl[b].T.astype(np.float16))
        m["corr"] = np.ascontiguousarray(corr[b].astype(np.float32))
        if flags[0]:
            m["corrb"] = np.ascontiguousarray(corrb[b].astype(np.float32))
        in_maps.append(m)
    return flags, in_maps


def kernel(**inputs):
    flags, in_maps = _prep_host(inputs)
    if flags not in _CACHE:
        _CACHE[flags] = _build_program(flags)
    nc = _CACHE[flags]
    res = bass_utils.run_bass_kernel_spmd(nc, in_maps, core_ids=list(range(B)))
    out = np.stack([res.results[b]["out"][0] for b in range(B)], axis=0)
    return out.astype(np.float32)
